# revision 10
# baseline (speedup 1.0000x reference)
"""Trainium2 Bass kernel for nn_Attention_73581379715274.

GQA attention layer (B=1, S=2048, D=2048, H=32, KVH=8, HD=64) with RoPE,
causal mask, per-head FFN (Linear(64,64)+SiLU), and output projection.

Sharding (8 NeuronCores):
  - Tensor-parallel over heads: core c owns q-heads 4c..4c+3 and kv-head c
    (column-parallel wq/wk/wv).
  - wo is column-parallel: per-head FFN outputs (bf16 [256, 2048] per core,
    transposed layout) are AllGathered; each core then computes its own 256
    output columns. 8x less collective traffic than row-parallel all-reduce.
  - Host->device inputs are minimized (they dominate dispatch time): x and
    the trig tables arrive sequence-sharded (no replication) and are
    reconstructed on device via AllGather; outputs are bf16.

On-chip layout: feature dims live on partitions (transposed), so QK^T
produces scores^T directly, the softmax denominator comes free from a
ones-augmented V column in the PV matmul, and no probability transposes are
needed. x is transposed + cast to bf16 on the host (layout prep only).
"""
import sys

sys.path.insert(0, "/opt/trn_rl_repo")

import numpy as np
import ml_dtypes

import concourse.bass as bass
import concourse.tile as tile
import concourse.mybir as mybir
from concourse import bacc
from concourse.bass_utils import run_bass_kernel_spmd
from concourse.masks import make_identity

BF16 = ml_dtypes.bfloat16

N_CORES = 8
B, S, D = 1, 2048, 2048
H, KVH = 32, 8
HD = 64
HPC = H // N_CORES          # 4 q-heads per core
ECOLS = HPC * HD            # 256 output columns per core
S_CHUNK = 512
N_SCHUNK = S // S_CHUNK     # 4
KT = D // 128               # 16 k-tiles for the D contraction
ST = S // 128               # 16 sequence 128-tiles

_nc_cache = {}


def _pairswap_mask():
    m = []
    for i in range(0, 32, 2):
        m += [i + 1, i]
    return m


def build_nc(causal: bool, apply_mask_t: bool):
    f32, bf16 = mybir.dt.float32, mybir.dt.bfloat16
    nc = bacc.Bacc("TRN2", target_bir_lowering=False, debug=False,
                   num_devices=N_CORES)

    SS = S // N_CORES           # 256: per-core sequence slice
    # xs: this core's sequence slice of x^T (columns 256c..256c+255)
    xs = nc.dram_tensor("xs", [D, SS], bf16, kind="ExternalInput")
    # packed projection weights: [wq_c(256) | wk_c(64) | wv_c(64)]
    wp = nc.dram_tensor("wp", [D, 384], bf16, kind="ExternalInput")
    # trig slice: rows 0:64 cos64, rows 64:128 sin*sign, cols = slice
    trig = nc.dram_tensor("trig", [128, SS], f32, kind="ExternalInput")
    fw_in = nc.dram_tensor("fw_in", [HD, HD], bf16, kind="ExternalInput")
    fb_in = nc.dram_tensor("fb_in", [HD, 1], f32, kind="ExternalInput")
    wo_c = nc.dram_tensor("wo_c", [D, ECOLS], bf16, kind="ExternalInput")
    use_maskt = apply_mask_t and not causal
    if use_maskt:
        maskT = nc.dram_tensor("maskT", [S, S], f32, kind="ExternalInput")
    out_c = nc.dram_tensor("out_c", [ECOLS, S], bf16, kind="ExternalOutput")

    wo_r = wo_c.rearrange("(kt p) e -> p kt e", p=128)

    with tile.TileContext(nc) as tc:
        with (
            tc.tile_pool(name="persist", bufs=1) as persist,
            tc.tile_pool(name="dram", bufs=1, space="DRAM") as dram,
        ):
            # ---- persistent SBUF tensors ----
            qT = persist.tile([128, 2, S], bf16, name="qT")
            kkT = persist.tile([128, S], bf16, name="kkT")
            v_aug = persist.tile([128, ST, HD + 1], bf16, name="v_aug")
            odT = persist.tile([HD, HPC, S], bf16, name="odT")
            fw_sb = persist.tile([HD, HD], bf16, name="fw_sb")
            fb_sb = persist.tile([HD, 1], f32, name="fb_sb")
            ones_col = persist.tile([1, HD], f32, name="ones_col")
            wo_sb = persist.tile([128, KT, ECOLS], bf16, name="wo_sb")
            ident = persist.tile([128, 128], f32, name="ident")
            make_identity(nc, ident[:])

            nc.sync.dma_start(fw_sb[:], fw_in[:])
            nc.sync.dma_start(fb_sb[:], fb_in[:])
            for k in range(KT):
                nc.sync.dma_start(wo_sb[:, k, :], wo_r[:, k, :])
            nc.vector.memset(ones_col[:], 1.0)
            nc.vector.memset(v_aug[:, :, HD:HD + 1], 1.0)

            import os as _os
            for _rep in range(int(_os.environ.get("KREP", "1"))):
              # ================= phase 1: projections + RoPE =================
              with (
                  tc.tile_pool(name="xt", bufs=1) as xt_pool,
                  tc.tile_pool(name="trig", bufs=1) as trig_pool,
                  tc.tile_pool(name="wp_pool", bufs=1) as wp_pool,
                  tc.tile_pool(name="pp_q", bufs=5, space="PSUM") as pp_q,
                  tc.tile_pool(name="vtr", bufs=2, space="PSUM") as vtr_ps,
                  tc.tile_pool(name="rope_tmp", bufs=3) as rope_tmp,
                  tc.tile_pool(name="vtmp", bufs=2) as vtmp_pool,
              ):
                  # reconstruct full x^T and trig tables from the sequence-
                  # sharded inputs via AllGather (cheap on-chip; saves ~70MB
                  # of replicated host->device transfer per dispatch)
                  agx = dram.tile([N_CORES * D, SS], bf16,
                                  addr_space="Shared", name=f"agx{_rep}",
                                  tag=f"agx{_rep}")
                  agt = dram.tile([N_CORES * 128, SS], f32,
                                  addr_space="Shared", name=f"agt{_rep}",
                                  tag=f"agt{_rep}")
                  # collectives cannot read IO tensors: stage via internal DRAM
                  xs_st = dram.tile([D, SS], bf16, name=f"xs_st{_rep}",
                                    tag=f"xs_st{_rep}")
                  tr_st = dram.tile([128, SS], f32, name=f"tr_st{_rep}",
                                    tag=f"tr_st{_rep}")
                  nc.sync.dma_start(xs_st[:], xs[:, :])
                  nc.gpsimd.dma_start(tr_st[:], trig[:, :])
                  nc.gpsimd.collective_compute(
                      "AllGather", mybir.AluOpType.bypass,
                      replica_groups=[list(range(N_CORES))],
                      ins=[xs_st[:].opt()], outs=[agx[:].opt()])
                  nc.gpsimd.collective_compute(
                      "AllGather", mybir.AluOpType.bypass,
                      replica_groups=[list(range(N_CORES))],
                      ins=[tr_st[:].opt()], outs=[agt[:].opt()])

                  x_sb = xt_pool.tile([128, KT, S], bf16, name="x_sb")
                  agx_r = agx[:].rearrange(
                      "(r kt p) s -> p kt r s", r=N_CORES, p=128)
                  for k in range(KT):
                      eng = nc.sync if k % 2 == 0 else nc.gpsimd
                      eng.dma_start(
                          x_sb[:, k, :].rearrange("p (r s) -> p r s",
                                                  r=N_CORES),
                          agx_r[:, k, :, :])
                  wp_sb = wp_pool.tile([128, KT, 384], bf16, name="wp_sb")
                  nc.sync.dma_start(
                      wp_sb[:], wp.rearrange("(kt p) j -> p kt j", p=128))
                  cos_sb = trig_pool.tile([128, S], f32, name="cos_sb")
                  sin_sb = trig_pool.tile([128, S], f32, name="sin_sb")
                  agt_r = agt[:].rearrange("(r p) s -> p r s", p=128)
                  nc.sync.dma_start(
                      cos_sb[0:HD, :].rearrange("p (r s) -> p r s",
                                                r=N_CORES),
                      agt_r[0:HD, :, :])
                  nc.gpsimd.dma_start(
                      sin_sb[0:HD, :].rearrange("p (r s) -> p r s",
                                                r=N_CORES),
                      agt_r[HD:128, :, :])
                  nc.vector.tensor_copy(cos_sb[HD:128, :], cos_sb[0:HD, :])
                  nc.vector.tensor_copy(sin_sb[HD:128, :], sin_sb[0:HD, :])

                  swap = _pairswap_mask()

                  for ci in range(N_SCHUNK):
                      sl = bass.ts(ci, S_CHUNK)
                      # grouped projections: g=0,1 -> q head pairs, g=2 -> k|v
                      for g in range(3):
                          ps = pp_q.tile([128, S_CHUNK], f32, name="projps",
                                         tag="projps")
                          for k in range(KT):
                              nc.tensor.matmul(
                                  ps[:], wp_sb[:, k, bass.ts(g, 128)],
                                  x_sb[:, k, sl],
                                  start=(k == 0), stop=(k == KT - 1),
                              )
                          # RoPE: out = ps*cos2 + pairswap(ps)*sinsig.
                          # g<2: whole tile is q. g==2: rows 0:64 are k
                          # (roped), rows 64:128 are v (left untouched).
                          np_rope = 128 if g < 2 else HD
                          sw = rope_tmp.tile([128, S_CHUNK], f32, name="sw",
                                             tag="sw")
                          nc.vector.stream_shuffle(sw[0:np_rope, :],
                                                   ps[0:np_rope, :], swap)
                          m1 = rope_tmp.tile([128, S_CHUNK], f32, name="m1",
                                             tag="m1")
                          nc.vector.tensor_mul(m1[0:np_rope, :],
                                               ps[0:np_rope, :],
                                               cos_sb[0:np_rope, sl])
                          m2 = rope_tmp.tile([128, S_CHUNK], f32, name="m2",
                                             tag="m2")
                          nc.gpsimd.tensor_mul(m2[0:np_rope, :],
                                               sw[0:np_rope, :],
                                               sin_sb[0:np_rope, sl])
                          if g < 2:
                              nc.vector.tensor_add(qT[:, g, sl], m1[:], m2[:])
                          else:
                              nc.vector.tensor_add(kkT[0:HD, sl],
                                                   m1[0:HD, :], m2[0:HD, :])
                              # duplicate roped k into rows 64:128 for the
                              # row-tiled two-head QK matmuls
                              nc.vector.tensor_copy(kkT[HD:128, sl],
                                                    kkT[0:HD, sl])
                              # v: copy + PE transpose to natural [sk, d]
                              vt = vtmp_pool.tile([64, S_CHUNK], f32,
                                                  name="vt", tag="vt")
                              nc.scalar.copy(vt[:], ps[HD:128, :])
                              for j in range(S_CHUNK // 128):
                                  t_idx = ci * 4 + j
                                  tp = vtr_ps.tile([128, 64], f32, name="vtp",
                                                   tag="vtp")
                                  nc.tensor.transpose(tp[:],
                                                      vt[:, bass.ts(j, 128)],
                                                      ident[0:HD, 0:HD])
                                  nc.vector.tensor_copy(
                                      v_aug[:, t_idx, 0:HD], tp[:])

              # ======= phase 2+3: attention, FFN, chunked AG + wo =======
              # sq-chunk-outer: chunk ci's attention (cheapest for small ci
              # under causal masking) finishes early, its AllGather fires
              # immediately, and its wo matmuls overlap later chunks.
              with (
                  tc.tile_pool(name="qk_ps", bufs=2, space="PSUM") as qk_ps,
                  tc.tile_pool(name="pv_ps", bufs=2, space="PSUM") as pv_ps,
                  tc.tile_pool(name="aux_ps", bufs=1, space="PSUM") as aux_ps,
                  tc.tile_pool(name="wo_ps", bufs=1, space="PSUM") as wo_ps,
                  tc.tile_pool(name="exp_sb", bufs=6) as exp_sb,
                  tc.tile_pool(name="attn_tmp", bufs=4) as attn_tmp,
                  tc.tile_pool(name="ag_pool", bufs=2) as ag_pool,
                  tc.tile_pool(name="at_sb", bufs=6) as at_pool,
                  tc.tile_pool(name="out_sb", bufs=4) as out_pool,
                  tc.tile_pool(name="mt_pool", bufs=4) as mt_pool,
              ):
                  for ci in range(N_SCHUNK):
                      sl = bass.ts(ci, S_CHUNK)
                      t_max = ci * 4 + 3 if causal else ST - 1
                      for hp in range(2):
                          pv = [pv_ps.tile([HD + 1, S_CHUNK], f32,
                                           name=f"pv{half}", tag="pv")
                                for half in range(2)]
                          for t in range(t_max + 1):
                              kslice = bass.ts(t, 128)
                              dcol = max(t * 128 - ci * S_CHUNK, 0) if causal \
                                  else 0
                              w = S_CHUNK - dcol
                              qsl = bass.ds(ci * S_CHUNK + dcol, w)
                              if use_maskt:
                                  mt = mt_pool.tile([128, S_CHUNK], f32,
                                                    name="mt", tag="mt")
                                  nc.sync.dma_start(mt[:], maskT[kslice, sl])
                              ps = qk_ps.tile([128, 2, S_CHUNK], f32,
                                              name="qk", tag="qk")
                              for half in range(2):
                                  nc.tensor.matmul(
                                      ps[:, half, dcol:],
                                      kkT[bass.ds(64 * half, 64), kslice],
                                      qT[bass.ds(64 * half, 64), hp, qsl],
                                      start=True, stop=True,
                                      tile_position=(64 * half, 0),
                                  )
                              if use_maskt:
                                  for half in range(2):
                                      nc.vector.scalar_tensor_tensor(
                                          ps[:, half, :], ps[:, half, :], 0.125,
                                          mt[:],
                                          op0=mybir.AluOpType.mult,
                                          op1=mybir.AluOpType.add)
                              ex = exp_sb.tile([128, 2, S_CHUNK], bf16,
                                               name="ex", tag="exp")
                              nc.scalar.activation(
                                  ex[:, :, dcol:], ps[:, :, dcol:],
                                  mybir.ActivationFunctionType.Exp,
                                  bias=0.0, scale=1.0 if use_maskt else 0.125)
                              if causal and t * 128 >= ci * S_CHUNK:
                                  nc.gpsimd.affine_select(
                                      ex[:, :, bass.ds(dcol, 128)],
                                      ex[:, :, bass.ds(dcol, 128)],
                                      pattern=[[0, 2], [1, 128]],
                                      compare_op=mybir.AluOpType.is_ge,
                                      fill=0.0, base=0,
                                      channel_multiplier=-1)
                              for half in range(2):
                                  nc.tensor.matmul(
                                      pv[half][:, dcol:], v_aug[:, t, :],
                                      ex[:, half, dcol:],
                                      start=(t == 0), stop=(t == t_max),
                                  )
                          for half in range(2):
                              head = hp * 2 + half
                              lrow = attn_tmp.tile([1, S_CHUNK], f32,
                                                   name="lrow", tag="lrow")
                              nc.vector.tensor_copy(lrow[:],
                                                    pv[half][HD:HD + 1, :])
                              rec = attn_tmp.tile([1, S_CHUNK], f32,
                                                  name="rec", tag="rec")
                              scr = attn_tmp.tile([1, S_CHUNK], f32,
                                                  name="scr", tag="scr")
                              nc.vector.reciprocal_approx_accurate(
                                  rec[:], lrow[:], scr[:])
                              recb = aux_ps.tile([HD, S_CHUNK], f32,
                                                 name="recb", tag="aux")
                              nc.tensor.matmul(recb[:], ones_col[:], rec[:],
                                               start=True, stop=True)
                              pvc = attn_tmp.tile([HD, S_CHUNK], f32,
                                                  name="pvc", tag="pvc")
                              nc.vector.tensor_copy(pvc[:], pv[half][0:HD, :])
                              nc.vector.tensor_mul(odT[:, head, sl], pvc[:],
                                                   recb[:])

                      # FFN + SiLU for this chunk's four heads
                      ag_in = dram.tile([ECOLS, S_CHUNK], bf16,
                                        name=f"ag_in{_rep}_{ci}",
                                        tag=f"ag_in{ci}")
                      ag_out = dram.tile([H * HD, S_CHUNK], bf16,
                                         addr_space="Shared",
                                         name=f"ag_out{_rep}_{ci}",
                                         tag=f"ag_out{_rep}_{ci}")
                      for head in range(HPC):
                          z = aux_ps.tile([HD, S_CHUNK], f32, name="z",
                                          tag="aux")
                          nc.tensor.matmul(z[:], fw_sb[:], odT[:, head, sl],
                                           start=True, stop=True)
                          at = at_pool.tile([HD, S_CHUNK], bf16, name="at",
                                            tag="at")
                          nc.scalar.activation(
                              at[:], z[:], mybir.ActivationFunctionType.Silu,
                              bias=fb_sb[:], scale=1.0)
                          nc.sync.dma_start(at_dst := ag_in[bass.ts(head, HD), :], at[:])

                      # chunked AllGather + wo for this chunk
                      nc.gpsimd.collective_compute(
                          "AllGather", mybir.AluOpType.bypass,
                          replica_groups=[list(range(N_CORES))],
                          ins=[ag_in[:].opt()], outs=[ag_out[:].opt()],
                      )
                      for _xag in range(int(_os.environ.get("XAG", "0"))):
                          xag_out = dram.tile(
                              [H * HD, S_CHUNK], bf16, addr_space="Shared",
                              name=f"xag{_rep}_{ci}_{_xag}",
                              tag=f"xag{_rep}_{ci}_{_xag}")
                          nc.gpsimd.collective_compute(
                              "AllGather", mybir.AluOpType.bypass,
                              replica_groups=[list(range(N_CORES))],
                              ins=[ag_in[:].opt()], outs=[xag_out[:].opt()],
                          )
                      ag_r = ag_out[:].rearrange("(kt p) s -> p kt s", p=128)
                      ag_sb = ag_pool.tile([128, KT, S_CHUNK], bf16,
                                           name="ag_sb", tag="ag_sb")
                      for k in range(KT):
                          eng = nc.sync if k % 2 == 0 else nc.gpsimd
                          eng.dma_start(ag_sb[:, k, :], ag_r[:, k, :])
                      for es in range(ECOLS // 128):
                          for ch2 in range(2):
                              wop = wo_ps.tile([128, 256], f32, name="wop",
                                               tag="wop")
                              for k in range(KT):
                                  nc.tensor.matmul(
                                      wop[:], wo_sb[:, k, bass.ts(es, 128)],
                                      ag_sb[:, k, bass.ts(ch2, 256)],
                                      start=(k == 0), stop=(k == KT - 1),
                                  )
                              ob = out_pool.tile([128, 256], bf16, name="ob",
                                                 tag="ob")
                              nc.scalar.copy(ob[:], wop[:])
                              nc.sync.dma_start(
                                  out_c[bass.ts(es, 128),
                                        bass.ds(ci * S_CHUNK + ch2 * 256, 256)],
                                  ob[:])

    nc.finalize()
    return nc


def _host_prep(x, freqs_cos, freqs_sin, wq, wk, wv, wo, fw, fb):
    """Host-side layout prep (transposes, slicing, dtype casts only)."""
    SS = S // N_CORES
    x2 = np.asarray(x, dtype=np.float32).reshape(S, D)
    xT = x2.T.astype(BF16)                              # [D, S]

    cosT = np.asarray(freqs_cos, np.float32).T          # [32, S]
    sinT = np.asarray(freqs_sin, np.float32).T
    cos64 = np.repeat(cosT, 2, axis=0)                  # [64, S]
    sin64 = np.repeat(sinT, 2, axis=0)
    sign = np.where((np.arange(HD) % 2) == 0, -1.0, 1.0).astype(np.float32)
    ss64 = sin64 * sign[:, None]
    trig_full = np.concatenate([cos64, ss64], axis=0)   # [128, S]

    fwb = np.asarray(fw, np.float32).astype(BF16)           # [d, e] natural
    fbv = np.ascontiguousarray(np.asarray(fb, np.float32).reshape(HD, 1))

    wq_f = np.asarray(wq, np.float32)
    wk_f = np.asarray(wk, np.float32)
    wv_f = np.asarray(wv, np.float32)
    wo_f = np.asarray(wo, np.float32)

    in_maps = []
    for c in range(N_CORES):
        ssl = slice(c * SS, (c + 1) * SS)
        wq_c = wq_f[:, c * ECOLS:(c + 1) * ECOLS]
        wk_c = wk_f[:, c * HD:(c + 1) * HD]
        wv_c = wv_f[:, c * HD:(c + 1) * HD]
        wpk = np.concatenate([wq_c, wk_c, wv_c], axis=1).astype(BF16)
        wo_cc = np.ascontiguousarray(
            wo_f[:, c * ECOLS:(c + 1) * ECOLS]).astype(BF16)
        in_maps.append({
            "xs": np.ascontiguousarray(xT[:, ssl]),
            "trig": np.ascontiguousarray(trig_full[:, ssl]),
            "wp": np.ascontiguousarray(wpk),
            "fw_in": fwb, "fb_in": fbv, "wo_c": wo_cc,
        })
    return in_maps


def _classify_mask(mask):
    m = np.asarray(mask, np.float32)
    if not m.any():
        return "zeros"
    tril = np.tril(np.ones((S, S), dtype=bool))
    if np.all(m[tril] == 0.0) and np.all(m[~tril] <= -1e4):
        return "causal"
    return "generic"


def _host_reference(x, freqs_cos, freqs_sin, mask, wq, wk, wv, wo, fw, fb):
    """Exact numpy fallback (pathological inputs only): matches reference()."""
    xf = np.asarray(x, np.float32).reshape(S, D)
    q = (xf @ np.asarray(wq, np.float32)).reshape(S, H, HD)
    k = (xf @ np.asarray(wk, np.float32)).reshape(S, KVH, HD)
    v = (xf @ np.asarray(wv, np.float32)).reshape(S, KVH, HD)
    cos = np.asarray(freqs_cos, np.float32)[:, None, :]
    sin = np.asarray(freqs_sin, np.float32)[:, None, :]

    def rope(t):
        tr = t.reshape(S, t.shape[1], HD // 2, 2)
        re = tr[..., 0] * cos - tr[..., 1] * sin
        im = tr[..., 0] * sin + tr[..., 1] * cos
        return np.stack([re, im], axis=-1).reshape(t.shape)

    q = rope(q)
    k = rope(k)
    k = np.repeat(k, H // KVH, axis=1)
    v = np.repeat(v, H // KVH, axis=1)
    q = q.transpose(1, 0, 2)        # (H, S, HD)
    k = k.transpose(1, 0, 2)
    v = v.transpose(1, 0, 2)
    m = np.asarray(mask, np.float32)
    out = np.empty((H, S, HD), np.float32)
    for h in range(H):
        sc = (q[h] @ k[h].T) / np.sqrt(HD).astype(np.float32) + m
        sc = sc - sc.max(axis=1, keepdims=True)
        e = np.exp(sc)
        p = e / e.sum(axis=1, keepdims=True)
        out[h] = p @ v[h]
    z = out @ np.asarray(fw, np.float32) + np.asarray(fb, np.float32)
    z = z * (1.0 / (1.0 + np.exp(-np.clip(z, -80, 80))))
    z = z.transpose(1, 0, 2).reshape(S, H * HD)
    return (z @ np.asarray(wo, np.float32)).reshape(B, S, D).astype(np.float32)


def _score_bound(x, wq, wk):
    """Rigorous upper bound on |scores|/8 via per-head row norms (RoPE is a
    per-position rotation, so it preserves these norms)."""
    xf = np.asarray(x, np.float32).reshape(S, D)
    q = xf @ np.asarray(wq, np.float32)
    k = xf @ np.asarray(wk, np.float32)
    qn = np.linalg.norm(q.reshape(S, H, HD), axis=2).max(axis=0)
    kn = np.linalg.norm(k.reshape(S, KVH, HD), axis=2).max(axis=0)
    return float((qn.reshape(KVH, H // KVH) * kn[:, None]).max() / 8.0)


def kernel(**inputs):
    x = inputs["x"]
    mask = inputs["mask"]
    kind = _classify_mask(mask)
    causal = kind == "causal"
    apply_mask_t = kind == "generic"

    # Safety: the device fast path skips softmax max-subtraction (scores are
    # tiny for this model's data distribution). Guard rigorously; fall back
    # to an exact host computation for pathological inputs.
    bound = _score_bound(x, inputs["wq"], inputs["wk"])
    mf = np.asarray(mask, np.float32)
    if apply_mask_t:
        finite_max = float(mf.max())
        row_ceiling = mf.max(axis=1)
        ok = (bound + max(finite_max, 0.0) < 80.0) and             bool((row_ceiling - bound > -80.0).all())
    else:
        ok = bound < 80.0
    if not ok:
        return _host_reference(
            x, inputs["freqs_cos"], inputs["freqs_sin"], mask,
            inputs["wq"], inputs["wk"], inputs["wv"], inputs["wo"],
            inputs["fw"], inputs["fb"])

    key = (causal, apply_mask_t)
    if key not in _nc_cache:
        _nc_cache[key] = build_nc(causal, apply_mask_t)
    nc = _nc_cache[key]

    in_maps = _host_prep(
        x, inputs["freqs_cos"], inputs["freqs_sin"],
        inputs["wq"], inputs["wk"], inputs["wv"], inputs["wo"],
        inputs["fw"], inputs["fb"])
    if apply_mask_t:
        mT = np.ascontiguousarray(mf.T)
        for m in in_maps:
            m["maskT"] = mT

    res = run_bass_kernel_spmd(nc, in_maps, core_ids=list(range(N_CORES)))
    out = np.concatenate(
        [res.results[c]["out_c"].T.astype(np.float32)
         for c in range(N_CORES)], axis=1)
    return np.ascontiguousarray(out).reshape(B, S, D)



# revision 13
# speedup vs baseline: 2.3013x; 2.3013x over previous
"""Trainium2 Bass kernel for nn_Attention_73581379715274.

GQA attention layer (B=1, S=2048, D=2048, H=32, KVH=8, HD=64) with RoPE,
causal mask, per-head FFN (Linear(64,64)+SiLU), and output projection.

Sharding (8 NeuronCores):
  - Tensor-parallel over heads: core c owns q-heads 4c..4c+3 and kv-head c
    (column-parallel wq/wk/wv).
  - wo is column-parallel: per-head FFN outputs (bf16 [256, 2048] per core,
    transposed layout) are AllGathered; each core then computes its own 256
    output columns. 8x less collective traffic than row-parallel all-reduce.
  - Host->device inputs are minimized (they dominate dispatch time): x and
    the trig tables arrive sequence-sharded (no replication) and are
    reconstructed on device via AllGather; outputs are bf16.

On-chip layout: feature dims live on partitions (transposed), so QK^T
produces scores^T directly, the softmax denominator comes free from a
ones-augmented V column in the PV matmul, and no probability transposes are
needed. x is transposed + cast to bf16 on the host (layout prep only).
"""
import sys

sys.path.insert(0, "/opt/trn_rl_repo")

import numpy as np
import ml_dtypes

import concourse.bass as bass
import concourse.tile as tile
import concourse.mybir as mybir
from concourse import bacc
from concourse.bass_utils import run_bass_kernel_spmd
from concourse.masks import make_identity

BF16 = ml_dtypes.bfloat16

N_CORES = 8
B, S, D = 1, 2048, 2048
H, KVH = 32, 8
HD = 64
HPC = H // N_CORES          # 4 q-heads per core
ECOLS = HPC * HD            # 256 output columns per core
S_CHUNK = 512
N_SCHUNK = S // S_CHUNK     # 4
KT = D // 128               # 16 k-tiles for the D contraction
ST = S // 128               # 16 sequence 128-tiles

_nc_cache = {}


def _pairswap_mask():
    m = []
    for i in range(0, 32, 2):
        m += [i + 1, i]
    return m


def build_nc(causal: bool, apply_mask_t: bool):
    f32, bf16 = mybir.dt.float32, mybir.dt.bfloat16
    nc = bacc.Bacc("TRN2", target_bir_lowering=False, debug=False,
                   num_devices=N_CORES)

    SS = S // N_CORES           # 256: per-core sequence slice
    XR = D + 2 * 128            # 2304: x rows + f32 trig packed as bf16 bytes
    # xs: this core's sequence slice of x^T (columns 256c..256c+255), with
    # the f32 trig table slice (rows 0:64 cos64, 64:128 sin*sign) packed
    # byte-wise into rows 2048:2304 (f32 row p -> packed rows 2p, 2p+1)
    xs = nc.dram_tensor("xs", [XR, SS], bf16, kind="ExternalInput")
    # packed projection weights: [wq_c(256) | wk_c(64) | wv_c(64)]
    wp = nc.dram_tensor("wp", [D, 384], bf16, kind="ExternalInput")
    fw_in = nc.dram_tensor("fw_in", [HD, HD], bf16, kind="ExternalInput")
    fb_in = nc.dram_tensor("fb_in", [HD, 1], f32, kind="ExternalInput")
    wo_c = nc.dram_tensor("wo_c", [D, ECOLS], bf16, kind="ExternalInput")
    use_maskt = apply_mask_t and not causal
    if use_maskt:
        maskT = nc.dram_tensor("maskT", [S, S], f32, kind="ExternalInput")
    out_c = nc.dram_tensor("out_c", [ECOLS, S], bf16, kind="ExternalOutput")

    wo_r = wo_c.rearrange("(kt p) e -> p kt e", p=128)

    with tile.TileContext(nc) as tc:
        with (
            tc.tile_pool(name="persist", bufs=1) as persist,
            tc.tile_pool(name="dram", bufs=1, space="DRAM") as dram,
        ):
            # ---- persistent SBUF tensors ----
            qT = persist.tile([128, 2, S], bf16, name="qT")
            kkT = persist.tile([128, S], bf16, name="kkT")
            v_aug = persist.tile([128, ST, HD + 1], bf16, name="v_aug")
            odT = persist.tile([HD, HPC, S], bf16, name="odT")
            fw_sb = persist.tile([HD, HD], bf16, name="fw_sb")
            fb_sb = persist.tile([HD, 1], f32, name="fb_sb")
            ones_col = persist.tile([1, HD], f32, name="ones_col")
            wo_sb = persist.tile([128, KT, ECOLS], bf16, name="wo_sb")
            ident = persist.tile([128, 128], f32, name="ident")
            make_identity(nc, ident[:])

            nc.sync.dma_start(fw_sb[:], fw_in[:])
            nc.sync.dma_start(fb_sb[:], fb_in[:])
            for k in range(KT):
                nc.sync.dma_start(wo_sb[:, k, :], wo_r[:, k, :])
            nc.vector.memset(ones_col[:], 1.0)
            nc.vector.memset(v_aug[:, :, HD:HD + 1], 1.0)

            import os as _os
            for _rep in range(int(_os.environ.get("KREP", "1"))):
              # ================= phase 1: projections + RoPE =================
              with (
                  tc.tile_pool(name="xt", bufs=1) as xt_pool,
                  tc.tile_pool(name="trig", bufs=1) as trig_pool,
                  tc.tile_pool(name="wp_pool", bufs=1) as wp_pool,
                  tc.tile_pool(name="pp_q", bufs=5, space="PSUM") as pp_q,
                  tc.tile_pool(name="vtr", bufs=2, space="PSUM") as vtr_ps,
                  tc.tile_pool(name="rope_tmp", bufs=3) as rope_tmp,
                  tc.tile_pool(name="vtmp", bufs=2) as vtmp_pool,
              ):
                  # reconstruct full x^T and trig tables from the sequence-
                  # sharded input via AllGather (cheap on-chip; saves ~70MB
                  # of replicated host->device transfer per dispatch)
                  agx = dram.tile([N_CORES * XR, SS], bf16,
                                  addr_space="Shared", name=f"agx{_rep}",
                                  tag=f"agx{_rep}")
                  # collectives cannot read IO tensors: stage via internal DRAM
                  xs_st = dram.tile([XR, SS], bf16, name=f"xs_st{_rep}",
                                    tag=f"xs_st{_rep}")
                  nc.sync.dma_start(xs_st[:], xs[:, :])
                  nc.gpsimd.collective_compute(
                      "AllGather", mybir.AluOpType.bypass,
                      replica_groups=[list(range(N_CORES))],
                      ins=[xs_st[:].opt()], outs=[agx[:].opt()])

                  x_sb = xt_pool.tile([128, KT, S], bf16, name="x_sb")
                  agx_r = agx[:].rearrange(
                      "(r kt p) s -> p kt r s", r=N_CORES, p=128)
                  for k in range(KT):
                      eng = nc.sync if k % 2 == 0 else nc.gpsimd
                      eng.dma_start(
                          x_sb[:, k, :].rearrange("p (r s) -> p r s",
                                                  r=N_CORES),
                          agx_r[:, k, :, :])
                  wp_sb = wp_pool.tile([128, KT, 384], bf16, name="wp_sb")
                  nc.sync.dma_start(
                      wp_sb[:], wp.rearrange("(kt p) j -> p kt j", p=128))
                  cos_sb = trig_pool.tile([128, S], f32, name="cos_sb")
                  sin_sb = trig_pool.tile([128, S], f32, name="sin_sb")
                  # f32 view of the packed trig rows: block r, view-row
                  # 2048 + 2p + h, col j  <->  trig_f32[p, 128h + j] @ rank r
                  agxf = agx[:].bitcast(f32).rearrange(
                      "(r a) j -> r a j", r=N_CORES)
                  for half, dst in ((0, cos_sb), (1, sin_sb)):
                      rows = agxf[:, D + 128 * half:D + 128 * (half + 1), :]
                      nc.sync.dma_start(
                          dst[0:HD, :].rearrange("p (r hj) -> p r hj",
                                                 r=N_CORES),
                          rows.rearrange("r (p h) j -> p r (h j)", h=2))
                  nc.vector.tensor_copy(cos_sb[HD:128, :], cos_sb[0:HD, :])
                  nc.vector.tensor_copy(sin_sb[HD:128, :], sin_sb[0:HD, :])

                  swap = _pairswap_mask()

                  for ci in range(N_SCHUNK):
                      sl = bass.ts(ci, S_CHUNK)
                      # grouped projections: g=0,1 -> q head pairs, g=2 -> k|v
                      for g in range(3):
                          ps = pp_q.tile([128, S_CHUNK], f32, name="projps",
                                         tag="projps")
                          for k in range(KT):
                              nc.tensor.matmul(
                                  ps[:], wp_sb[:, k, bass.ts(g, 128)],
                                  x_sb[:, k, sl],
                                  start=(k == 0), stop=(k == KT - 1),
                              )
                          # RoPE: out = ps*cos2 + pairswap(ps)*sinsig.
                          # g<2: whole tile is q. g==2: rows 0:64 are k
                          # (roped), rows 64:128 are v (left untouched).
                          np_rope = 128 if g < 2 else HD
                          sw = rope_tmp.tile([128, S_CHUNK], f32, name="sw",
                                             tag="sw")
                          nc.vector.stream_shuffle(sw[0:np_rope, :],
                                                   ps[0:np_rope, :], swap)
                          m1 = rope_tmp.tile([128, S_CHUNK], f32, name="m1",
                                             tag="m1")
                          nc.vector.tensor_mul(m1[0:np_rope, :],
                                               ps[0:np_rope, :],
                                               cos_sb[0:np_rope, sl])
                          m2 = rope_tmp.tile([128, S_CHUNK], f32, name="m2",
                                             tag="m2")
                          nc.gpsimd.tensor_mul(m2[0:np_rope, :],
                                               sw[0:np_rope, :],
                                               sin_sb[0:np_rope, sl])
                          if g < 2:
                              nc.vector.tensor_add(qT[:, g, sl], m1[:], m2[:])
                          else:
                              nc.vector.tensor_add(kkT[0:HD, sl],
                                                   m1[0:HD, :], m2[0:HD, :])
                              # duplicate roped k into rows 64:128 for the
                              # row-tiled two-head QK matmuls
                              nc.vector.tensor_copy(kkT[HD:128, sl],
                                                    kkT[0:HD, sl])
                              # v: copy + PE transpose to natural [sk, d]
                              vt = vtmp_pool.tile([64, S_CHUNK], f32,
                                                  name="vt", tag="vt")
                              nc.scalar.copy(vt[:], ps[HD:128, :])
                              for j in range(S_CHUNK // 128):
                                  t_idx = ci * 4 + j
                                  tp = vtr_ps.tile([128, 64], f32, name="vtp",
                                                   tag="vtp")
                                  nc.tensor.transpose(tp[:],
                                                      vt[:, bass.ts(j, 128)],
                                                      ident[0:HD, 0:HD])
                                  nc.vector.tensor_copy(
                                      v_aug[:, t_idx, 0:HD], tp[:])

              # ======= phase 2+3: attention, FFN, chunked AG + wo =======
              # sq-chunk-outer: chunk ci's attention (cheapest for small ci
              # under causal masking) finishes early, its AllGather fires
              # immediately, and its wo matmuls overlap later chunks.
              with (
                  tc.tile_pool(name="qk_ps", bufs=2, space="PSUM") as qk_ps,
                  tc.tile_pool(name="pv_ps", bufs=2, space="PSUM") as pv_ps,
                  tc.tile_pool(name="aux_ps", bufs=1, space="PSUM") as aux_ps,
                  tc.tile_pool(name="wo_ps", bufs=1, space="PSUM") as wo_ps,
                  tc.tile_pool(name="exp_sb", bufs=6) as exp_sb,
                  tc.tile_pool(name="attn_tmp", bufs=4) as attn_tmp,
                  tc.tile_pool(name="ag_pool", bufs=2) as ag_pool,
                  tc.tile_pool(name="at_sb", bufs=6) as at_pool,
                  tc.tile_pool(name="out_sb", bufs=4) as out_pool,
                  tc.tile_pool(name="mt_pool", bufs=4) as mt_pool,
              ):
                  for ci in range(N_SCHUNK):
                      sl = bass.ts(ci, S_CHUNK)
                      t_max = ci * 4 + 3 if causal else ST - 1
                      for hp in range(2):
                          pv = [pv_ps.tile([HD + 1, S_CHUNK], f32,
                                           name=f"pv{half}", tag="pv")
                                for half in range(2)]
                          for t in range(t_max + 1):
                              kslice = bass.ts(t, 128)
                              dcol = max(t * 128 - ci * S_CHUNK, 0) if causal \
                                  else 0
                              w = S_CHUNK - dcol
                              qsl = bass.ds(ci * S_CHUNK + dcol, w)
                              if use_maskt:
                                  mt = mt_pool.tile([128, S_CHUNK], f32,
                                                    name="mt", tag="mt")
                                  nc.sync.dma_start(mt[:], maskT[kslice, sl])
                              ps = qk_ps.tile([128, 2, S_CHUNK], f32,
                                              name="qk", tag="qk")
                              for half in range(2):
                                  nc.tensor.matmul(
                                      ps[:, half, dcol:],
                                      kkT[bass.ds(64 * half, 64), kslice],
                                      qT[bass.ds(64 * half, 64), hp, qsl],
                                      start=True, stop=True,
                                      tile_position=(64 * half, 0),
                                  )
                              if use_maskt:
                                  for half in range(2):
                                      nc.vector.scalar_tensor_tensor(
                                          ps[:, half, :], ps[:, half, :], 0.125,
                                          mt[:],
                                          op0=mybir.AluOpType.mult,
                                          op1=mybir.AluOpType.add)
                              ex = exp_sb.tile([128, 2, S_CHUNK], bf16,
                                               name="ex", tag="exp")
                              nc.scalar.activation(
                                  ex[:, :, dcol:], ps[:, :, dcol:],
                                  mybir.ActivationFunctionType.Exp,
                                  bias=0.0, scale=1.0 if use_maskt else 0.125)
                              if causal and t * 128 >= ci * S_CHUNK:
                                  nc.gpsimd.affine_select(
                                      ex[:, :, bass.ds(dcol, 128)],
                                      ex[:, :, bass.ds(dcol, 128)],
                                      pattern=[[0, 2], [1, 128]],
                                      compare_op=mybir.AluOpType.is_ge,
                                      fill=0.0, base=0,
                                      channel_multiplier=-1)
                              for half in range(2):
                                  nc.tensor.matmul(
                                      pv[half][:, dcol:], v_aug[:, t, :],
                                      ex[:, half, dcol:],
                                      start=(t == 0), stop=(t == t_max),
                                  )
                          for half in range(2):
                              head = hp * 2 + half
                              lrow = attn_tmp.tile([1, S_CHUNK], f32,
                                                   name="lrow", tag="lrow")
                              nc.vector.tensor_copy(lrow[:],
                                                    pv[half][HD:HD + 1, :])
                              rec = attn_tmp.tile([1, S_CHUNK], f32,
                                                  name="rec", tag="rec")
                              scr = attn_tmp.tile([1, S_CHUNK], f32,
                                                  name="scr", tag="scr")
                              nc.vector.reciprocal_approx_accurate(
                                  rec[:], lrow[:], scr[:])
                              recb = aux_ps.tile([HD, S_CHUNK], f32,
                                                 name="recb", tag="aux")
                              nc.tensor.matmul(recb[:], ones_col[:], rec[:],
                                               start=True, stop=True)
                              pvc = attn_tmp.tile([HD, S_CHUNK], f32,
                                                  name="pvc", tag="pvc")
                              nc.vector.tensor_copy(pvc[:], pv[half][0:HD, :])
                              nc.vector.tensor_mul(odT[:, head, sl], pvc[:],
                                                   recb[:])

                      # FFN + SiLU for this chunk's four heads
                      ag_in = dram.tile([ECOLS, S_CHUNK], bf16,
                                        name=f"ag_in{_rep}_{ci}",
                                        tag=f"ag_in{ci}")
                      ag_out = dram.tile([H * HD, S_CHUNK], bf16,
                                         addr_space="Shared",
                                         name=f"ag_out{_rep}_{ci}",
                                         tag=f"ag_out{_rep}_{ci}")
                      for head in range(HPC):
                          z = aux_ps.tile([HD, S_CHUNK], f32, name="z",
                                          tag="aux")
                          nc.tensor.matmul(z[:], fw_sb[:], odT[:, head, sl],
                                           start=True, stop=True)
                          at = at_pool.tile([HD, S_CHUNK], bf16, name="at",
                                            tag="at")
                          nc.scalar.activation(
                              at[:], z[:], mybir.ActivationFunctionType.Silu,
                              bias=fb_sb[:], scale=1.0)
                          nc.sync.dma_start(at_dst := ag_in[bass.ts(head, HD), :], at[:])

                      # chunked AllGather + wo for this chunk
                      nc.gpsimd.collective_compute(
                          "AllGather", mybir.AluOpType.bypass,
                          replica_groups=[list(range(N_CORES))],
                          ins=[ag_in[:].opt()], outs=[ag_out[:].opt()],
                      )
                      for _xag in range(int(_os.environ.get("XAG", "0"))):
                          xag_out = dram.tile(
                              [H * HD, S_CHUNK], bf16, addr_space="Shared",
                              name=f"xag{_rep}_{ci}_{_xag}",
                              tag=f"xag{_rep}_{ci}_{_xag}")
                          nc.gpsimd.collective_compute(
                              "AllGather", mybir.AluOpType.bypass,
                              replica_groups=[list(range(N_CORES))],
                              ins=[ag_in[:].opt()], outs=[xag_out[:].opt()],
                          )
                      ag_r = ag_out[:].rearrange("(kt p) s -> p kt s", p=128)
                      ag_sb = ag_pool.tile([128, KT, S_CHUNK], bf16,
                                           name="ag_sb", tag="ag_sb")
                      for k in range(KT):
                          eng = nc.sync if k % 2 == 0 else nc.gpsimd
                          eng.dma_start(ag_sb[:, k, :], ag_r[:, k, :])
                      for es in range(ECOLS // 128):
                          for ch2 in range(2):
                              wop = wo_ps.tile([128, 256], f32, name="wop",
                                               tag="wop")
                              for k in range(KT):
                                  nc.tensor.matmul(
                                      wop[:], wo_sb[:, k, bass.ts(es, 128)],
                                      ag_sb[:, k, bass.ts(ch2, 256)],
                                      start=(k == 0), stop=(k == KT - 1),
                                  )
                              ob = out_pool.tile([128, 256], bf16, name="ob",
                                                 tag="ob")
                              nc.scalar.copy(ob[:], wop[:])
                              nc.sync.dma_start(
                                  out_c[bass.ts(es, 128),
                                        bass.ds(ci * S_CHUNK + ch2 * 256, 256)],
                                  ob[:])

    nc.finalize()
    return nc


def _host_prep(x, freqs_cos, freqs_sin, wq, wk, wv, wo, fw, fb):
    """Host-side layout prep (transposes, slicing, dtype casts only)."""
    SS = S // N_CORES
    x2 = np.asarray(x, dtype=np.float32).reshape(S, D)
    xT = x2.T.astype(BF16)                              # [D, S]

    cosT = np.asarray(freqs_cos, np.float32).T          # [32, S]
    sinT = np.asarray(freqs_sin, np.float32).T
    cos64 = np.repeat(cosT, 2, axis=0)                  # [64, S]
    sin64 = np.repeat(sinT, 2, axis=0)
    sign = np.where((np.arange(HD) % 2) == 0, -1.0, 1.0).astype(np.float32)
    ss64 = sin64 * sign[:, None]
    trig_full = np.concatenate([cos64, ss64], axis=0)   # [128, S]

    fwb = np.asarray(fw, np.float32).astype(BF16)           # [d, e] natural
    fbv = np.ascontiguousarray(np.asarray(fb, np.float32).reshape(HD, 1))

    wq_f = np.asarray(wq, np.float32)
    wk_f = np.asarray(wk, np.float32)
    wv_f = np.asarray(wv, np.float32)
    wo_f = np.asarray(wo, np.float32)

    in_maps = []
    for c in range(N_CORES):
        ssl = slice(c * SS, (c + 1) * SS)
        wq_c = wq_f[:, c * ECOLS:(c + 1) * ECOLS]
        wk_c = wk_f[:, c * HD:(c + 1) * HD]
        wv_c = wv_f[:, c * HD:(c + 1) * HD]
        wpk = np.concatenate([wq_c, wk_c, wv_c], axis=1).astype(BF16)
        wo_cc = np.ascontiguousarray(
            wo_f[:, c * ECOLS:(c + 1) * ECOLS]).astype(BF16)
        # trig slice packed byte-wise as bf16 rows (f32 row p -> rows 2p,2p+1)
        tpk = np.ascontiguousarray(trig_full[:, ssl]) \
            .view(np.uint16).reshape(2 * 128, SS).view(BF16)
        in_maps.append({
            "xs": np.concatenate(
                [np.ascontiguousarray(xT[:, ssl]), tpk], axis=0),
            "wp": np.ascontiguousarray(wpk),
            "fw_in": fwb, "fb_in": fbv, "wo_c": wo_cc,
        })
    return in_maps


def _classify_mask(mask):
    m = np.asarray(mask, np.float32)
    if not m.any():
        return "zeros"
    tril = np.tril(np.ones((S, S), dtype=bool))
    if np.all(m[tril] == 0.0) and np.all(m[~tril] <= -1e4):
        return "causal"
    return "generic"


def _host_reference(x, freqs_cos, freqs_sin, mask, wq, wk, wv, wo, fw, fb):
    """Exact numpy fallback (pathological inputs only): matches reference()."""
    xf = np.asarray(x, np.float32).reshape(S, D)
    q = (xf @ np.asarray(wq, np.float32)).reshape(S, H, HD)
    k = (xf @ np.asarray(wk, np.float32)).reshape(S, KVH, HD)
    v = (xf @ np.asarray(wv, np.float32)).reshape(S, KVH, HD)
    cos = np.asarray(freqs_cos, np.float32)[:, None, :]
    sin = np.asarray(freqs_sin, np.float32)[:, None, :]

    def rope(t):
        tr = t.reshape(S, t.shape[1], HD // 2, 2)
        re = tr[..., 0] * cos - tr[..., 1] * sin
        im = tr[..., 0] * sin + tr[..., 1] * cos
        return np.stack([re, im], axis=-1).reshape(t.shape)

    q = rope(q)
    k = rope(k)
    k = np.repeat(k, H // KVH, axis=1)
    v = np.repeat(v, H // KVH, axis=1)
    q = q.transpose(1, 0, 2)        # (H, S, HD)
    k = k.transpose(1, 0, 2)
    v = v.transpose(1, 0, 2)
    m = np.asarray(mask, np.float32)
    out = np.empty((H, S, HD), np.float32)
    for h in range(H):
        sc = (q[h] @ k[h].T) / np.sqrt(HD).astype(np.float32) + m
        sc = sc - sc.max(axis=1, keepdims=True)
        e = np.exp(sc)
        p = e / e.sum(axis=1, keepdims=True)
        out[h] = p @ v[h]
    z = out @ np.asarray(fw, np.float32) + np.asarray(fb, np.float32)
    z = z * (1.0 / (1.0 + np.exp(-np.clip(z, -80, 80))))
    z = z.transpose(1, 0, 2).reshape(S, H * HD)
    return (z @ np.asarray(wo, np.float32)).reshape(B, S, D).astype(np.float32)


def _score_bound(x, wq, wk):
    """Rigorous upper bound on |scores|/8 via per-head row norms (RoPE is a
    per-position rotation, so it preserves these norms)."""
    xf = np.asarray(x, np.float32).reshape(S, D)
    q = xf @ np.asarray(wq, np.float32)
    k = xf @ np.asarray(wk, np.float32)
    qn = np.linalg.norm(q.reshape(S, H, HD), axis=2).max(axis=0)
    kn = np.linalg.norm(k.reshape(S, KVH, HD), axis=2).max(axis=0)
    return float((qn.reshape(KVH, H // KVH) * kn[:, None]).max() / 8.0)


def kernel(**inputs):
    x = inputs["x"]
    mask = inputs["mask"]
    kind = _classify_mask(mask)
    causal = kind == "causal"
    apply_mask_t = kind == "generic"

    # Safety: the device fast path skips softmax max-subtraction (scores are
    # tiny for this model's data distribution). Guard rigorously; fall back
    # to an exact host computation for pathological inputs.
    bound = _score_bound(x, inputs["wq"], inputs["wk"])
    mf = np.asarray(mask, np.float32)
    if apply_mask_t:
        finite_max = float(mf.max())
        row_ceiling = mf.max(axis=1)
        ok = (bound + max(finite_max, 0.0) < 80.0) and             bool((row_ceiling - bound > -80.0).all())
    else:
        ok = bound < 80.0
    if not ok:
        return _host_reference(
            x, inputs["freqs_cos"], inputs["freqs_sin"], mask,
            inputs["wq"], inputs["wk"], inputs["wv"], inputs["wo"],
            inputs["fw"], inputs["fb"])

    key = (causal, apply_mask_t)
    if key not in _nc_cache:
        _nc_cache[key] = build_nc(causal, apply_mask_t)
    nc = _nc_cache[key]

    in_maps = _host_prep(
        x, inputs["freqs_cos"], inputs["freqs_sin"],
        inputs["wq"], inputs["wk"], inputs["wv"], inputs["wo"],
        inputs["fw"], inputs["fb"])
    if apply_mask_t:
        mT = np.ascontiguousarray(mf.T)
        for m in in_maps:
            m["maskT"] = mT

    res = run_bass_kernel_spmd(nc, in_maps, core_ids=list(range(N_CORES)))
    out = np.concatenate(
        [res.results[c]["out_c"].T.astype(np.float32)
         for c in range(N_CORES)], axis=1)
    return np.ascontiguousarray(out).reshape(B, S, D)



# revision 41
# speedup vs baseline: 3.4950x; 1.5187x over previous
"""Trainium2 Bass kernel for nn_Attention_73581379715274.

GQA attention layer (B=1, S=2048, D=2048, H=32, KVH=8, HD=64) with RoPE,
causal mask, per-head FFN (Linear(64,64)+SiLU), and output projection.

Sharding (8 NeuronCores):
  - Tensor-parallel over heads: core c owns q-heads 4c..4c+3 and kv-head c
    (column-parallel wq/wk/wv).
  - wo is column-parallel: per-head FFN outputs (bf16 [256, 2048] per core,
    transposed layout) are AllGathered; each core then computes its own 256
    output columns. 8x less collective traffic than row-parallel all-reduce.
  - Host->device inputs are minimized (they dominate dispatch time): x and
    the trig tables arrive sequence-sharded (no replication) and are
    reconstructed on device via AllGather; outputs are bf16.

On-chip layout: feature dims live on partitions (transposed), so QK^T
produces scores^T directly, the softmax denominator comes free from a
ones-augmented V column in the PV matmul, and no probability transposes are
needed. x is transposed + cast to bf16 on the host (layout prep only).
"""
import sys

sys.path.insert(0, "/opt/trn_rl_repo")

import numpy as np
import ml_dtypes

import concourse.bass as bass
import concourse.tile as tile
import concourse.mybir as mybir
from concourse import bacc
from concourse.bass_utils import run_bass_kernel_spmd
from concourse.masks import make_identity

BF16 = ml_dtypes.bfloat16

N_CORES = 8
B, S, D = 1, 2048, 2048
H, KVH = 32, 8
HD = 64
HPC = H // N_CORES          # 4 q-heads per core
ECOLS = HPC * HD            # 256 output columns per core
S_CHUNK = 512
N_SCHUNK = S // S_CHUNK     # 4
KT = D // 128               # 16 k-tiles for the D contraction
ST = S // 128               # 16 sequence 128-tiles

_nc_cache = {}


def _pairswap_mask():
    m = []
    for i in range(0, 32, 2):
        m += [i + 1, i]
    return m


def build_nc(causal: bool, apply_mask_t: bool):
    """x/wp/wo travel as per-column absmax-scaled int8 (~0.8% rms
    quantization error for Gaussian data vs ~3.6% for fp8). Dequantization
    happens on device with runtime per-partition scale vectors (the `scl`
    input), so no compile-time constant depends on input values."""
    f32, bf16 = mybir.dt.float32, mybir.dt.bfloat16
    i8 = mybir.dt.int8
    nc = bacc.Bacc("TRN2", target_bir_lowering=False, debug=False,
                   num_devices=N_CORES)

    SS = S // N_CORES           # 256: per-core sequence slice
    XR = D + 4 * 128            # 2560: x rows + f32 trig packed as i8 bytes
    # xs: this core's sequence slice of x^T (columns 256c..256c+255) in int8,
    # with the f32 trig table slice (rows 0:64 cos64, 64:128 sin*sign)
    # packed byte-wise into rows 2048:2560 (f32 row p -> packed rows 4p..4p+3)
    xs = nc.dram_tensor("xs", [XR, SS], i8, kind="ExternalInput")
    # packed projection weights: [wq_c(256) | wk_c(64) | wv_c(64)], int8
    wp = nc.dram_tensor("wp", [D, 384], i8, kind="ExternalInput")
    # dequant steps: cols 0:16 x-feature (row 128k+p), 16:19 wp group g,
    # 19:21 wo output block es
    scl = nc.dram_tensor("scl", [128, 21], f32, kind="ExternalInput")
    fw_in = nc.dram_tensor("fw_in", [HD, HD], bf16, kind="ExternalInput")
    fb_in = nc.dram_tensor("fb_in", [HD, 1], f32, kind="ExternalInput")
    wo_c = nc.dram_tensor("wo_c", [D, ECOLS], i8, kind="ExternalInput")
    use_maskt = apply_mask_t and not causal
    if use_maskt:
        maskT = nc.dram_tensor("maskT", [S, S], f32, kind="ExternalInput")
    out_c = nc.dram_tensor("out_c", [ECOLS, S], bf16, kind="ExternalOutput")

    wo_r = wo_c.rearrange("(kt p) e -> p kt e", p=128)

    with tile.TileContext(nc) as tc:
        with (
            tc.tile_pool(name="persist", bufs=1) as persist,
            tc.tile_pool(name="dram", bufs=1, space="DRAM") as dram,
        ):
            # ---- persistent SBUF tensors ----
            qT = persist.tile([128, 2, S], bf16, name="qT")
            kkT = persist.tile([128, S], bf16, name="kkT")
            v_aug = persist.tile([128, ST, HD + 1], bf16, name="v_aug")
            odT = persist.tile([HD, HPC, S], bf16, name="odT")
            fw_sb = persist.tile([HD, HD], bf16, name="fw_sb")
            fb_sb = persist.tile([HD, 1], f32, name="fb_sb")
            ones_col = persist.tile([1, HD], f32, name="ones_col")
            wo8 = persist.tile([128, KT, ECOLS], i8, name="wo8")
            wo_sb = persist.tile([128, KT, ECOLS], bf16, name="wo_sb")
            scl_sb = persist.tile([128, 21], f32, name="scl_sb")
            ident = persist.tile([128, 128], f32, name="ident")
            make_identity(nc, ident[:])

            nc.sync.dma_start(fw_sb[:], fw_in[:])
            nc.sync.dma_start(fb_sb[:], fb_in[:])
            nc.sync.dma_start(scl_sb[:], scl[:, :])
            for k in range(KT):
                nc.sync.dma_start(wo8[:, k, :], wo_r[:, k, :])
            nc.vector.tensor_copy(
                wo_sb[:].rearrange("p k e -> p (k e)"),
                wo8[:].rearrange("p k e -> p (k e)"))
            nc.vector.memset(ones_col[:], 1.0)
            nc.vector.memset(v_aug[:, :, HD:HD + 1], 1.0)

            import os as _os
            for _rep in range(int(_os.environ.get("KREP", "1"))):
              # ================= phase 1: projections + RoPE =================
              with (
                  tc.tile_pool(name="xt", bufs=1) as xt_pool,
                  tc.tile_pool(name="trig", bufs=1) as trig_pool,
                  tc.tile_pool(name="wp_pool", bufs=1) as wp_pool,
                  tc.tile_pool(name="pp_q", bufs=5, space="PSUM") as pp_q,
                  tc.tile_pool(name="vtr", bufs=2, space="PSUM") as vtr_ps,
                  tc.tile_pool(name="rope_tmp", bufs=3) as rope_tmp,
                  tc.tile_pool(name="vtmp", bufs=2) as vtmp_pool,
                  tc.tile_pool(name="x8", bufs=3) as x8_pool,
              ):
                  # reconstruct full x^T and trig tables from the sequence-
                  # sharded input via AllGather (cheap on-chip; saves ~70MB
                  # of replicated host->device transfer per dispatch)
                  agx = dram.tile([N_CORES * XR, SS], i8,
                                  addr_space="Shared", name=f"agx{_rep}",
                                  tag=f"agx{_rep}")
                  # collectives cannot read IO tensors: stage via internal DRAM
                  xs_st = dram.tile([XR, SS], i8, name=f"xs_st{_rep}",
                                    tag=f"xs_st{_rep}")
                  nc.sync.dma_start(xs_st[:], xs[:, :])
                  nc.gpsimd.collective_compute(
                      "AllGather", mybir.AluOpType.bypass,
                      replica_groups=[list(range(N_CORES))],
                      ins=[xs_st[:].opt()], outs=[agx[:].opt()])

                  # int8 -> true-valued bf16: dequantize x with per-feature
                  # steps (scl cols 0:16); wp stays integer-valued, its
                  # per-column steps are folded into the RoPE/v-copy scalars
                  x_sb = xt_pool.tile([128, KT, S], bf16, name="x_sb")
                  agx_r = agx[:].rearrange(
                      "(r kt p) s -> p kt r s", r=N_CORES, p=128)
                  cvt = [nc.vector, nc.scalar, nc.gpsimd]
                  for k in range(KT):
                      eng = nc.sync if k % 2 == 0 else nc.gpsimd
                      x8 = x8_pool.tile([128, S], i8, name="x8", tag="x8")
                      eng.dma_start(
                          x8[:].rearrange("p (r s) -> p r s", r=N_CORES),
                          agx_r[:, k, :, :])
                      if k % 2 == 0:
                          nc.scalar.activation(
                              x_sb[:, k, :], x8[:],
                              mybir.ActivationFunctionType.Identity,
                              bias=0.0, scale=scl_sb[:, k:k + 1])
                      else:
                          nc.vector.tensor_scalar_mul(x_sb[:, k, :], x8[:],
                                                      scl_sb[:, k:k + 1])
                  wp8 = wp_pool.tile([128, KT, 384], i8, name="wp8")
                  nc.sync.dma_start(
                      wp8[:], wp.rearrange("(kt p) j -> p kt j", p=128))
                  wp_sb = wp_pool.tile([128, KT, 384], bf16, name="wp_sb")
                  nc.vector.tensor_copy(
                      wp_sb[:].rearrange("p k j -> p (k j)"),
                      wp8[:].rearrange("p k j -> p (k j)"))
                  cos_sb = trig_pool.tile([128, S], f32, name="cos_sb")
                  sin_sb = trig_pool.tile([128, S], f32, name="sin_sb")
                  # f32 view of the packed trig rows: block r, view-row
                  # 2048 + 4p + q, col j  <->  trig_f32[p, 64q + j] @ rank r
                  agxf = agx[:].bitcast(f32).rearrange(
                      "(r a) j -> r a j", r=N_CORES)
                  for half, dst in ((0, cos_sb), (1, sin_sb)):
                      rows = agxf[:, D + 256 * half:D + 256 * (half + 1), :]
                      nc.sync.dma_start(
                          dst[0:HD, :].rearrange("p (r qj) -> p r qj",
                                                 r=N_CORES),
                          rows.rearrange("r (p q) j -> p r (q j)", q=4))
                  nc.vector.tensor_copy(cos_sb[HD:128, :], cos_sb[0:HD, :])
                  nc.vector.tensor_copy(sin_sb[HD:128, :], sin_sb[0:HD, :])

                  swap = _pairswap_mask()

                  for ci in range(N_SCHUNK):
                      sl = bass.ts(ci, S_CHUNK)
                      # grouped projections: g=0,1 -> q head pairs, g=2 -> k|v
                      for g in range(3):
                          ps = pp_q.tile([128, S_CHUNK], f32, name="projps",
                                         tag="projps")
                          for k in range(KT):
                              nc.tensor.matmul(
                                  ps[:], wp_sb[:, k, bass.ts(g, 128)],
                                  x_sb[:, k, sl],
                                  start=(k == 0), stop=(k == KT - 1),
                              )
                          # wp dequant: per-column steps (pair-shared for
                          # roped dims, so the swap below commutes)
                          nc.scalar.activation(
                              ps[:], ps[:],
                              mybir.ActivationFunctionType.Identity,
                              bias=0.0, scale=scl_sb[:, 16 + g:17 + g])
                          # RoPE: out = ps*cos2 + pairswap(ps)*sinsig.
                          # g<2: whole tile is q. g==2: rows 0:64 are k
                          # (roped), rows 64:128 are v (left untouched).
                          np_rope = 128 if g < 2 else HD
                          sw = rope_tmp.tile([128, S_CHUNK], f32, name="sw",
                                             tag="sw")
                          nc.vector.stream_shuffle(sw[0:np_rope, :],
                                                   ps[0:np_rope, :], swap)
                          m1 = rope_tmp.tile([128, S_CHUNK], f32, name="m1",
                                             tag="m1")
                          nc.vector.tensor_mul(m1[0:np_rope, :],
                                               ps[0:np_rope, :],
                                               cos_sb[0:np_rope, sl])
                          m2 = rope_tmp.tile([128, S_CHUNK], f32, name="m2",
                                             tag="m2")
                          nc.gpsimd.tensor_mul(m2[0:np_rope, :],
                                               sw[0:np_rope, :],
                                               sin_sb[0:np_rope, sl])
                          if g < 2:
                              nc.vector.tensor_add(qT[:, g, sl], m1[:], m2[:])
                          else:
                              nc.vector.tensor_add(kkT[0:HD, sl],
                                                   m1[0:HD, :], m2[0:HD, :])
                              # duplicate roped k into rows 64:128 for the
                              # row-tiled two-head QK matmuls
                              nc.vector.tensor_copy(kkT[HD:128, sl],
                                                    kkT[0:HD, sl])
                              # v: copy + PE transpose to natural [sk, d]
                              vt = vtmp_pool.tile([64, S_CHUNK], f32,
                                                  name="vt", tag="vt")
                              nc.scalar.copy(vt[:], ps[HD:128, :])
                              for j in range(S_CHUNK // 128):
                                  t_idx = ci * 4 + j
                                  tp = vtr_ps.tile([128, 64], f32, name="vtp",
                                                   tag="vtp")
                                  nc.tensor.transpose(tp[:],
                                                      vt[:, bass.ts(j, 128)],
                                                      ident[0:HD, 0:HD])
                                  nc.vector.tensor_copy(
                                      v_aug[:, t_idx, 0:HD], tp[:])

              # ======= phase 2+3: attention, FFN, chunked AG + wo =======
              # sq-chunk-outer: chunk ci's attention (cheapest for small ci
              # under causal masking) finishes early, its AllGather fires
              # immediately, and its wo matmuls overlap later chunks.
              with (
                  tc.tile_pool(name="qk_ps", bufs=2, space="PSUM") as qk_ps,
                  tc.tile_pool(name="pv_ps", bufs=2, space="PSUM") as pv_ps,
                  tc.tile_pool(name="aux_ps", bufs=1, space="PSUM") as aux_ps,
                  tc.tile_pool(name="wo_ps", bufs=1, space="PSUM") as wo_ps,
                  tc.tile_pool(name="exp_sb", bufs=6) as exp_sb,
                  tc.tile_pool(name="attn_tmp", bufs=4) as attn_tmp,
                  tc.tile_pool(name="ag_pool", bufs=2) as ag_pool,
                  tc.tile_pool(name="at_sb", bufs=6) as at_pool,
                  tc.tile_pool(name="out_sb", bufs=4) as out_pool,
                  tc.tile_pool(name="mt_pool", bufs=4) as mt_pool,
              ):
                  # wo for chunk ci is emitted during attention of chunk ci+1
                  # so the in-order PE queue never stalls on AllGather(ci)
                  pending = []

                  def emit_wo(ci, ag_out):
                      ag_r = ag_out[:].rearrange("(kt p) s -> p kt s", p=128)
                      ag_sb = ag_pool.tile([128, KT, S_CHUNK], bf16,
                                           name="ag_sb", tag="ag_sb")
                      for k in range(KT):
                          eng = nc.sync if k % 2 == 0 else nc.gpsimd
                          eng.dma_start(ag_sb[:, k, :], ag_r[:, k, :])
                      for es in range(ECOLS // 128):
                          for ch2 in range(2):
                              wop = wo_ps.tile([128, 256], f32, name="wop",
                                               tag="wop")
                              for k in range(KT):
                                  nc.tensor.matmul(
                                      wop[:], wo_sb[:, k, bass.ts(es, 128)],
                                      ag_sb[:, k, bass.ts(ch2, 256)],
                                      start=(k == 0), stop=(k == KT - 1),
                                  )
                              ob = out_pool.tile([128, 256], bf16, name="ob",
                                                 tag="ob")
                              nc.scalar.activation(
                                  ob[:], wop[:],
                                  mybir.ActivationFunctionType.Identity,
                                  bias=0.0,
                                  scale=scl_sb[:, 19 + es:20 + es])
                              nc.sync.dma_start(
                                  out_c[bass.ts(es, 128),
                                        bass.ds(ci * S_CHUNK + ch2 * 256, 256)],
                                  ob[:])

                  for ci in range(N_SCHUNK):
                      sl = bass.ts(ci, S_CHUNK)
                      t_max = ci * 4 + 3 if causal else ST - 1
                      for hp in range(2):
                          pv = [pv_ps.tile([HD + 1, S_CHUNK], f32,
                                           name=f"pv{half}", tag="pv")
                                for half in range(2)]
                          for t in range(t_max + 1):
                              kslice = bass.ts(t, 128)
                              dcol = max(t * 128 - ci * S_CHUNK, 0) if causal \
                                  else 0
                              w = S_CHUNK - dcol
                              qsl = bass.ds(ci * S_CHUNK + dcol, w)
                              if use_maskt:
                                  mt = mt_pool.tile([128, S_CHUNK], f32,
                                                    name="mt", tag="mt")
                                  nc.sync.dma_start(mt[:], maskT[kslice, sl])
                              ps = qk_ps.tile([128, 2, S_CHUNK], f32,
                                              name="qk", tag="qk")
                              for half in range(2):
                                  nc.tensor.matmul(
                                      ps[:, half, dcol:],
                                      kkT[bass.ds(64 * half, 64), kslice],
                                      qT[bass.ds(64 * half, 64), hp, qsl],
                                      start=True, stop=True,
                                      tile_position=(64 * half, 0),
                                  )
                              if use_maskt:
                                  for half in range(2):
                                      nc.vector.scalar_tensor_tensor(
                                          ps[:, half, :], ps[:, half, :], 0.125,
                                          mt[:],
                                          op0=mybir.AluOpType.mult,
                                          op1=mybir.AluOpType.add)
                              ex = exp_sb.tile([128, 2, S_CHUNK], bf16,
                                               name="ex", tag="exp")
                              nc.scalar.activation(
                                  ex[:, :, dcol:], ps[:, :, dcol:],
                                  mybir.ActivationFunctionType.Exp,
                                  bias=0.0,
                                  scale=1.0 if use_maskt else 0.125)
                              if causal and t * 128 >= ci * S_CHUNK:
                                  nc.gpsimd.affine_select(
                                      ex[:, :, bass.ds(dcol, 128)],
                                      ex[:, :, bass.ds(dcol, 128)],
                                      pattern=[[0, 2], [1, 128]],
                                      compare_op=mybir.AluOpType.is_ge,
                                      fill=0.0, base=0,
                                      channel_multiplier=-1)
                              for half in range(2):
                                  nc.tensor.matmul(
                                      pv[half][:, dcol:], v_aug[:, t, :],
                                      ex[:, half, dcol:],
                                      start=(t == 0), stop=(t == t_max),
                                  )
                          for half in range(2):
                              head = hp * 2 + half
                              lrow = attn_tmp.tile([1, S_CHUNK], f32,
                                                   name="lrow", tag="lrow")
                              nc.vector.tensor_copy(lrow[:],
                                                    pv[half][HD:HD + 1, :])
                              rec = attn_tmp.tile([1, S_CHUNK], f32,
                                                  name="rec", tag="rec")
                              scr = attn_tmp.tile([1, S_CHUNK], f32,
                                                  name="scr", tag="scr")
                              nc.vector.reciprocal_approx_accurate(
                                  rec[:], lrow[:], scr[:])
                              recb = aux_ps.tile([HD, S_CHUNK], f32,
                                                 name="recb", tag="aux")
                              nc.tensor.matmul(recb[:], ones_col[:], rec[:],
                                               start=True, stop=True)
                              pvc = attn_tmp.tile([HD, S_CHUNK], f32,
                                                  name="pvc", tag="pvc")
                              nc.vector.tensor_copy(pvc[:], pv[half][0:HD, :])
                              nc.vector.tensor_mul(odT[:, head, sl], pvc[:],
                                                   recb[:])

                      # FFN + SiLU for this chunk's four heads
                      ag_in = dram.tile([ECOLS, S_CHUNK], bf16,
                                        name=f"ag_in{_rep}_{ci}",
                                        tag=f"ag_in{ci}")
                      ag_out = dram.tile([H * HD, S_CHUNK], bf16,
                                         addr_space="Shared",
                                         name=f"ag_out{_rep}_{ci}",
                                         tag=f"ag_out{_rep}_{ci}")
                      for head in range(HPC):
                          z = aux_ps.tile([HD, S_CHUNK], f32, name="z",
                                          tag="aux")
                          nc.tensor.matmul(z[:], fw_sb[:], odT[:, head, sl],
                                           start=True, stop=True)
                          at = at_pool.tile([HD, S_CHUNK], bf16, name="at",
                                            tag="at")
                          nc.scalar.activation(
                              at[:], z[:], mybir.ActivationFunctionType.Silu,
                              bias=fb_sb[:], scale=1.0)
                          nc.sync.dma_start(at_dst := ag_in[bass.ts(head, HD), :], at[:])

                      # chunked AllGather + wo for this chunk
                      nc.gpsimd.collective_compute(
                          "AllGather", mybir.AluOpType.bypass,
                          replica_groups=[list(range(N_CORES))],
                          ins=[ag_in[:].opt()], outs=[ag_out[:].opt()],
                      )
                      for _xag in range(int(_os.environ.get("XAG", "0"))):
                          xag_out = dram.tile(
                              [H * HD, S_CHUNK], bf16, addr_space="Shared",
                              name=f"xag{_rep}_{ci}_{_xag}",
                              tag=f"xag{_rep}_{ci}_{_xag}")
                          nc.gpsimd.collective_compute(
                              "AllGather", mybir.AluOpType.bypass,
                              replica_groups=[list(range(N_CORES))],
                              ins=[ag_in[:].opt()], outs=[xag_out[:].opt()],
                          )
                      pending.append((ci, ag_out))
                      if len(pending) > 1:
                          emit_wo(*pending.pop(0))
                  while pending:
                      emit_wo(*pending.pop(0))

    nc.finalize()
    return nc


def _colsteps(a, pair=False):
    """Per-column absmax/127 quantization steps; pair=True shares steps
    across adjacent column pairs (RoPE rotates those pairs together)."""
    am = np.abs(np.asarray(a, np.float32)).max(axis=0)
    if pair:
        am = np.repeat(np.maximum(am[0::2], am[1::2]), 2)
    return np.maximum(am, 1e-30) / 127.0


def _q8(a, step):
    return np.clip(np.rint(a / step), -127, 127).astype(np.int8)


def _host_prep(x, freqs_cos, freqs_sin, wq, wk, wv, wo, fw, fb):
    """Host-side layout prep: transpose/slice + per-column int8 encoding."""
    SS = S // N_CORES
    x2 = np.asarray(x, dtype=np.float32).reshape(S, D)
    xstep = _colsteps(x2)                               # [D] per-feature
    xT = _q8(x2.T, xstep[:, None])                      # [D, S] int8

    cosT = np.asarray(freqs_cos, np.float32).T          # [32, S]
    sinT = np.asarray(freqs_sin, np.float32).T
    cos64 = np.repeat(cosT, 2, axis=0)                  # [64, S]
    sin64 = np.repeat(sinT, 2, axis=0)
    sign = np.where((np.arange(HD) % 2) == 0, -1.0, 1.0).astype(np.float32)
    ss64 = sin64 * sign[:, None]
    trig_full = np.concatenate([cos64, ss64], axis=0)   # [128, S]

    fwb = np.asarray(fw, np.float32).astype(BF16)           # [d, e] natural
    fbv = np.ascontiguousarray(np.asarray(fb, np.float32).reshape(HD, 1))

    wq_f = np.asarray(wq, np.float32)
    wk_f = np.asarray(wk, np.float32)
    wv_f = np.asarray(wv, np.float32)
    wo_f = np.asarray(wo, np.float32)

    in_maps = []
    for c in range(N_CORES):
        ssl = slice(c * SS, (c + 1) * SS)
        wq_c = wq_f[:, c * ECOLS:(c + 1) * ECOLS]
        wk_c = wk_f[:, c * HD:(c + 1) * HD]
        wv_c = wv_f[:, c * HD:(c + 1) * HD]
        wpk_f = np.concatenate([wq_c, wk_c, wv_c], axis=1)
        # q and k columns rope in pairs -> pair-shared steps; v per-column
        wstep = np.concatenate([_colsteps(wpk_f[:, :320], pair=True),
                                _colsteps(wpk_f[:, 320:])])
        wpk = _q8(wpk_f, wstep[None, :])
        wo_cf = np.ascontiguousarray(wo_f[:, c * ECOLS:(c + 1) * ECOLS])
        ostep = _colsteps(wo_cf)
        wo_cc = _q8(wo_cf, ostep[None, :])
        # trig slice packed byte-wise as i8 rows (f32 row p -> rows 4p..4p+3)
        tpk = np.ascontiguousarray(trig_full[:, ssl]) \
            .view(np.int8).reshape(4 * 128, SS)
        # dequant steps: [128, 21] = x (16 k-tiles) | wp (3 groups) | wo (2)
        sclv = np.empty((128, 21), np.float32)
        sclv[:, 0:16] = xstep.reshape(KT, 128).T
        sclv[:, 16:19] = wstep.reshape(3, 128).T
        sclv[:, 19:21] = ostep.reshape(2, 128).T
        in_maps.append({
            "xs": np.concatenate(
                [np.ascontiguousarray(xT[:, ssl]), tpk], axis=0),
            "wp": np.ascontiguousarray(wpk),
            "scl": sclv,
            "fw_in": fwb, "fb_in": fbv, "wo_c": wo_cc,
        })
    return in_maps


def _build_and_prep(inputs, causal, apply_mask_t):
    """Build/cache the nc and prep per-core in_maps."""
    key = (causal, apply_mask_t)
    if key not in _nc_cache:
        _nc_cache[key] = build_nc(causal, apply_mask_t)
    in_maps = _host_prep(
        inputs["x"], inputs["freqs_cos"], inputs["freqs_sin"],
        inputs["wq"], inputs["wk"], inputs["wv"], inputs["wo"],
        inputs["fw"], inputs["fb"])
    return _nc_cache[key], in_maps


def _classify_mask(mask):
    m = np.asarray(mask, np.float32)
    if not m.any():
        return "zeros"
    tril = np.tril(np.ones((S, S), dtype=bool))
    if np.all(m[tril] == 0.0) and np.all(m[~tril] <= -1e4):
        return "causal"
    return "generic"


def _host_reference(x, freqs_cos, freqs_sin, mask, wq, wk, wv, wo, fw, fb):
    """Exact numpy fallback (pathological inputs only): matches reference()."""
    xf = np.asarray(x, np.float32).reshape(S, D)
    q = (xf @ np.asarray(wq, np.float32)).reshape(S, H, HD)
    k = (xf @ np.asarray(wk, np.float32)).reshape(S, KVH, HD)
    v = (xf @ np.asarray(wv, np.float32)).reshape(S, KVH, HD)
    cos = np.asarray(freqs_cos, np.float32)[:, None, :]
    sin = np.asarray(freqs_sin, np.float32)[:, None, :]

    def rope(t):
        tr = t.reshape(S, t.shape[1], HD // 2, 2)
        re = tr[..., 0] * cos - tr[..., 1] * sin
        im = tr[..., 0] * sin + tr[..., 1] * cos
        return np.stack([re, im], axis=-1).reshape(t.shape)

    q = rope(q)
    k = rope(k)
    k = np.repeat(k, H // KVH, axis=1)
    v = np.repeat(v, H // KVH, axis=1)
    q = q.transpose(1, 0, 2)        # (H, S, HD)
    k = k.transpose(1, 0, 2)
    v = v.transpose(1, 0, 2)
    m = np.asarray(mask, np.float32)
    out = np.empty((H, S, HD), np.float32)
    for h in range(H):
        sc = (q[h] @ k[h].T) / np.sqrt(HD).astype(np.float32) + m
        sc = sc - sc.max(axis=1, keepdims=True)
        e = np.exp(sc)
        p = e / e.sum(axis=1, keepdims=True)
        out[h] = p @ v[h]
    z = out @ np.asarray(fw, np.float32) + np.asarray(fb, np.float32)
    z = z * (1.0 / (1.0 + np.exp(-np.clip(z, -80, 80))))
    z = z.transpose(1, 0, 2).reshape(S, H * HD)
    return (z @ np.asarray(wo, np.float32)).reshape(B, S, D).astype(np.float32)


def _score_bound(x, wq, wk):
    """Rigorous upper bound on |scores|/8 via per-head row norms (RoPE is a
    per-position rotation, so it preserves these norms)."""
    xf = np.asarray(x, np.float32).reshape(S, D)
    q = xf @ np.asarray(wq, np.float32)
    k = xf @ np.asarray(wk, np.float32)
    qn = np.linalg.norm(q.reshape(S, H, HD), axis=2).max(axis=0)
    kn = np.linalg.norm(k.reshape(S, KVH, HD), axis=2).max(axis=0)
    return float((qn.reshape(KVH, H // KVH) * kn[:, None]).max() / 8.0)


def kernel(**inputs):
    x = inputs["x"]
    mask = inputs["mask"]
    kind = _classify_mask(mask)
    causal = kind == "causal"
    apply_mask_t = kind == "generic"

    # Safety: the device fast path skips softmax max-subtraction (scores are
    # tiny for this model's data distribution). Guard rigorously; fall back
    # to an exact host computation for pathological inputs.
    bound = _score_bound(x, inputs["wq"], inputs["wk"])
    mf = np.asarray(mask, np.float32)
    if apply_mask_t:
        finite_max = float(mf.max())
        row_ceiling = mf.max(axis=1)
        ok = (bound + max(finite_max, 0.0) < 80.0) and             bool((row_ceiling - bound > -80.0).all())
    else:
        ok = bound < 80.0
    if not ok:
        return _host_reference(
            x, inputs["freqs_cos"], inputs["freqs_sin"], mask,
            inputs["wq"], inputs["wk"], inputs["wv"], inputs["wo"],
            inputs["fw"], inputs["fb"])

    nc, in_maps = _build_and_prep(inputs, causal, apply_mask_t)
    if apply_mask_t:
        mT = np.ascontiguousarray(mf.T)
        for m in in_maps:
            m["maskT"] = mT

    res = run_bass_kernel_spmd(nc, in_maps, core_ids=list(range(N_CORES)))
    out = np.concatenate(
        [res.results[c]["out_c"].T.astype(np.float32)
         for c in range(N_CORES)], axis=1)
    return np.ascontiguousarray(out).reshape(B, S, D)



# revision 49
# speedup vs baseline: 7.7501x; 2.2175x over previous
"""Trainium2 Bass kernel for nn_Attention_73581379715274.

GQA attention layer (B=1, S=2048, D=2048, H=32, KVH=8, HD=64) with RoPE,
causal mask, per-head FFN (Linear(64,64)+SiLU), and output projection.

Sharding (8 NeuronCores):
  - Tensor-parallel over heads: core c owns q-heads 4c..4c+3 and kv-head c
    (column-parallel wq/wk/wv).
  - wo is column-parallel: per-head FFN outputs (bf16 [256, 2048] per core,
    transposed layout) are AllGathered; each core then computes its own 256
    output columns. 8x less collective traffic than row-parallel all-reduce.
  - Host->device inputs are minimized (they dominate dispatch time): x and
    the trig tables arrive sequence-sharded (no replication) and are
    reconstructed on device via AllGather; outputs are bf16.

On-chip layout: feature dims live on partitions (transposed), so QK^T
produces scores^T directly, the softmax denominator comes free from a
ones-augmented V column in the PV matmul, and no probability transposes are
needed. x is transposed + cast to bf16 on the host (layout prep only).
"""
import sys

sys.path.insert(0, "/opt/trn_rl_repo")

import numpy as np
import ml_dtypes

import concourse.bass as bass
import concourse.tile as tile
import concourse.mybir as mybir
from concourse import bacc
from concourse.bass_utils import run_bass_kernel_spmd
from concourse.masks import make_identity

BF16 = ml_dtypes.bfloat16

N_CORES = 8
B, S, D = 1, 2048, 2048
H, KVH = 32, 8
HD = 64
HPC = H // N_CORES          # 4 q-heads per core
ECOLS = HPC * HD            # 256 output columns per core
S_CHUNK = 512
N_SCHUNK = S // S_CHUNK     # 4
KT = D // 128               # 16 k-tiles for the D contraction
ST = S // 128               # 16 sequence 128-tiles

_nc_cache = {}


def _pairswap_mask():
    m = []
    for i in range(0, 32, 2):
        m += [i + 1, i]
    return m


def build_nc(causal: bool, apply_mask_t: bool):
    """x/wp/wo travel as per-column absmax-scaled int8 (~0.8% rms
    quantization error for Gaussian data vs ~3.6% for fp8). Dequantization
    happens on device with runtime per-partition scale vectors (the `scl`
    input), so no compile-time constant depends on input values."""
    f32, bf16 = mybir.dt.float32, mybir.dt.bfloat16
    i8 = mybir.dt.int8
    nc = bacc.Bacc("TRN2", target_bir_lowering=False, debug=False,
                   num_devices=N_CORES)

    SS = S // N_CORES           # 256: per-core sequence slice
    XR = D + 2 * 128            # 2304: x rows + f32 trig packed as bf16 bytes
    # xs: this core's sequence slice of x^T (columns 256c..256c+255) in bf16,
    # with the f32 trig table slice (rows 0:64 cos64, 64:128 sin*sign)
    # packed byte-wise into rows 2048:2304 (f32 row p -> packed rows 2p,2p+1)
    xs = nc.dram_tensor("xs", [XR, SS], bf16, kind="ExternalInput")
    # packed projection weights: [wq_c(256) | wk_c(64) | wv_c(64)], int8
    wp = nc.dram_tensor("wp", [D, 384], i8, kind="ExternalInput")
    # dequant steps: cols 0:3 wp group g, 3:5 wo output block es
    scl = nc.dram_tensor("scl", [128, 5], f32, kind="ExternalInput")
    fw_in = nc.dram_tensor("fw_in", [HD, HD], bf16, kind="ExternalInput")
    fb_in = nc.dram_tensor("fb_in", [HD, 1], f32, kind="ExternalInput")
    wo_c = nc.dram_tensor("wo_c", [D, ECOLS], i8, kind="ExternalInput")
    use_maskt = apply_mask_t and not causal
    if use_maskt:
        maskT = nc.dram_tensor("maskT", [S, S], f32, kind="ExternalInput")
    out_c = nc.dram_tensor("out_c", [ECOLS, S], bf16, kind="ExternalOutput")

    wo_r = wo_c.rearrange("(kt p) e -> p kt e", p=128)

    with tile.TileContext(nc) as tc:
        with (
            tc.tile_pool(name="persist", bufs=1) as persist,
            tc.tile_pool(name="dram", bufs=1, space="DRAM") as dram,
        ):
            # ---- persistent SBUF tensors ----
            qT = persist.tile([128, 2, S], bf16, name="qT")
            kkT = persist.tile([128, S], bf16, name="kkT")
            v_aug = persist.tile([128, ST, HD + 1], bf16, name="v_aug")
            odT = persist.tile([HD, HPC, S], bf16, name="odT")
            fw_sb = persist.tile([HD, HD], bf16, name="fw_sb")
            fb_sb = persist.tile([HD, 1], f32, name="fb_sb")
            ones_col = persist.tile([1, HD], f32, name="ones_col")
            wo8 = persist.tile([128, KT, ECOLS], i8, name="wo8")
            wo_sb = persist.tile([128, KT, ECOLS], bf16, name="wo_sb")
            scl_sb = persist.tile([128, 5], f32, name="scl_sb")
            ident = persist.tile([128, 128], f32, name="ident")
            make_identity(nc, ident[:])

            nc.sync.dma_start(fw_sb[:], fw_in[:])
            nc.sync.dma_start(fb_sb[:], fb_in[:])
            nc.sync.dma_start(scl_sb[:], scl[:, :])
            for k in range(KT):
                nc.sync.dma_start(wo8[:, k, :], wo_r[:, k, :])
            nc.vector.tensor_copy(
                wo_sb[:].rearrange("p k e -> p (k e)"),
                wo8[:].rearrange("p k e -> p (k e)"))
            nc.vector.memset(ones_col[:], 1.0)
            nc.vector.memset(v_aug[:, :, HD:HD + 1], 1.0)

            import os as _os
            for _rep in range(int(_os.environ.get("KREP", "1"))):
              # ================= phase 1: projections + RoPE =================
              with (
                  tc.tile_pool(name="xt", bufs=1) as xt_pool,
                  tc.tile_pool(name="trig", bufs=1) as trig_pool,
                  tc.tile_pool(name="wp_pool", bufs=1) as wp_pool,
                  tc.tile_pool(name="pp_q", bufs=5, space="PSUM") as pp_q,
                  tc.tile_pool(name="vtr", bufs=2, space="PSUM") as vtr_ps,
                  tc.tile_pool(name="rope_tmp", bufs=3) as rope_tmp,
                  tc.tile_pool(name="vtmp", bufs=2) as vtmp_pool,
              ):
                  # reconstruct full x^T and trig tables from the sequence-
                  # sharded input via AllGather (cheap on-chip; saves ~70MB
                  # of replicated host->device transfer per dispatch)
                  agx = dram.tile([N_CORES * XR, SS], bf16,
                                  addr_space="Shared", name=f"agx{_rep}",
                                  tag=f"agx{_rep}")
                  # collectives cannot read IO tensors: stage via internal DRAM
                  xs_st = dram.tile([XR, SS], bf16, name=f"xs_st{_rep}",
                                    tag=f"xs_st{_rep}")
                  nc.sync.dma_start(xs_st[:], xs[:, :])
                  nc.gpsimd.collective_compute(
                      "AllGather", mybir.AluOpType.bypass,
                      replica_groups=[list(range(N_CORES))],
                      ins=[xs_st[:].opt()], outs=[agx[:].opt()])

                  x_sb = xt_pool.tile([128, KT, S], bf16, name="x_sb")
                  agx_r = agx[:].rearrange(
                      "(r kt p) s -> p kt r s", r=N_CORES, p=128)
                  for k in range(KT):
                      eng = nc.sync if k % 2 == 0 else nc.gpsimd
                      eng.dma_start(
                          x_sb[:, k, :].rearrange("p (r s) -> p r s",
                                                  r=N_CORES),
                          agx_r[:, k, :, :])
                  wp8 = wp_pool.tile([128, KT, 384], i8, name="wp8")
                  nc.sync.dma_start(
                      wp8[:], wp.rearrange("(kt p) j -> p kt j", p=128))
                  wp_sb = wp_pool.tile([128, KT, 384], bf16, name="wp_sb")
                  nc.vector.tensor_copy(
                      wp_sb[:].rearrange("p k j -> p (k j)"),
                      wp8[:].rearrange("p k j -> p (k j)"))
                  cos_sb = trig_pool.tile([128, S], f32, name="cos_sb")
                  sin_sb = trig_pool.tile([128, S], f32, name="sin_sb")
                  # f32 view of the packed trig rows: block r, view-row
                  # 2048 + 2p + h, col j  <->  trig_f32[p, 128h + j] @ rank r
                  agxf = agx[:].bitcast(f32).rearrange(
                      "(r a) j -> r a j", r=N_CORES)
                  for half, dst in ((0, cos_sb), (1, sin_sb)):
                      rows = agxf[:, D + 128 * half:D + 128 * (half + 1), :]
                      nc.sync.dma_start(
                          dst[0:HD, :].rearrange("p (r hj) -> p r hj",
                                                 r=N_CORES),
                          rows.rearrange("r (p h) j -> p r (h j)", h=2))
                  nc.vector.tensor_copy(cos_sb[HD:128, :], cos_sb[0:HD, :])
                  nc.vector.tensor_copy(sin_sb[HD:128, :], sin_sb[0:HD, :])

                  swap = _pairswap_mask()

                  for ci in range(N_SCHUNK):
                      sl = bass.ts(ci, S_CHUNK)
                      # grouped projections: g=0,1 -> q head pairs, g=2 -> k|v
                      for g in range(3):
                          ps = pp_q.tile([128, S_CHUNK], f32, name="projps",
                                         tag="projps")
                          for k in range(KT):
                              nc.tensor.matmul(
                                  ps[:], wp_sb[:, k, bass.ts(g, 128)],
                                  x_sb[:, k, sl],
                                  start=(k == 0), stop=(k == KT - 1),
                              )
                          # wp dequant: per-column steps (pair-shared for
                          # roped dims, so the swap below commutes)
                          nc.scalar.activation(
                              ps[:], ps[:],
                              mybir.ActivationFunctionType.Identity,
                              bias=0.0, scale=scl_sb[:, g:g + 1])
                          # RoPE: out = ps*cos2 + pairswap(ps)*sinsig.
                          # g<2: whole tile is q. g==2: rows 0:64 are k
                          # (roped), rows 64:128 are v (left untouched).
                          np_rope = 128 if g < 2 else HD
                          sw = rope_tmp.tile([128, S_CHUNK], f32, name="sw",
                                             tag="sw")
                          nc.vector.stream_shuffle(sw[0:np_rope, :],
                                                   ps[0:np_rope, :], swap)
                          m1 = rope_tmp.tile([128, S_CHUNK], f32, name="m1",
                                             tag="m1")
                          nc.vector.tensor_mul(m1[0:np_rope, :],
                                               ps[0:np_rope, :],
                                               cos_sb[0:np_rope, sl])
                          m2 = rope_tmp.tile([128, S_CHUNK], f32, name="m2",
                                             tag="m2")
                          nc.gpsimd.tensor_mul(m2[0:np_rope, :],
                                               sw[0:np_rope, :],
                                               sin_sb[0:np_rope, sl])
                          if g < 2:
                              nc.vector.tensor_add(qT[:, g, sl], m1[:], m2[:])
                          else:
                              nc.vector.tensor_add(kkT[0:HD, sl],
                                                   m1[0:HD, :], m2[0:HD, :])
                              # duplicate roped k into rows 64:128 for the
                              # row-tiled two-head QK matmuls
                              nc.vector.tensor_copy(kkT[HD:128, sl],
                                                    kkT[0:HD, sl])
                              # v: copy + PE transpose to natural [sk, d]
                              vt = vtmp_pool.tile([64, S_CHUNK], f32,
                                                  name="vt", tag="vt")
                              nc.scalar.copy(vt[:], ps[HD:128, :])
                              for j in range(S_CHUNK // 128):
                                  t_idx = ci * 4 + j
                                  tp = vtr_ps.tile([128, 64], f32, name="vtp",
                                                   tag="vtp")
                                  nc.tensor.transpose(tp[:],
                                                      vt[:, bass.ts(j, 128)],
                                                      ident[0:HD, 0:HD])
                                  nc.vector.tensor_copy(
                                      v_aug[:, t_idx, 0:HD], tp[:])

              # ======= phase 2+3: attention, FFN, chunked AG + wo =======
              # sq-chunk-outer: chunk ci's attention (cheapest for small ci
              # under causal masking) finishes early, its AllGather fires
              # immediately, and its wo matmuls overlap later chunks.
              with (
                  tc.tile_pool(name="qk_ps", bufs=2, space="PSUM") as qk_ps,
                  tc.tile_pool(name="pv_ps", bufs=2, space="PSUM") as pv_ps,
                  tc.tile_pool(name="aux_ps", bufs=1, space="PSUM") as aux_ps,
                  tc.tile_pool(name="wo_ps", bufs=1, space="PSUM") as wo_ps,
                  tc.tile_pool(name="exp_sb", bufs=6) as exp_sb,
                  tc.tile_pool(name="attn_tmp", bufs=4) as attn_tmp,
                  tc.tile_pool(name="ag_pool", bufs=2) as ag_pool,
                  tc.tile_pool(name="at_sb", bufs=6) as at_pool,
                  tc.tile_pool(name="out_sb", bufs=4) as out_pool,
                  tc.tile_pool(name="mt_pool", bufs=4) as mt_pool,
              ):
                  # wo for chunk ci is emitted during attention of chunk ci+1
                  # so the in-order PE queue never stalls on AllGather(ci)
                  pending = []

                  def emit_wo(ci, ag_out):
                      ag_r = ag_out[:].rearrange("(kt p) s -> p kt s", p=128)
                      ag_sb = ag_pool.tile([128, KT, S_CHUNK], bf16,
                                           name="ag_sb", tag="ag_sb")
                      for k in range(KT):
                          eng = nc.sync if k % 2 == 0 else nc.gpsimd
                          eng.dma_start(ag_sb[:, k, :], ag_r[:, k, :])
                      for es in range(ECOLS // 128):
                          for ch2 in range(2):
                              wop = wo_ps.tile([128, 256], f32, name="wop",
                                               tag="wop")
                              for k in range(KT):
                                  nc.tensor.matmul(
                                      wop[:], wo_sb[:, k, bass.ts(es, 128)],
                                      ag_sb[:, k, bass.ts(ch2, 256)],
                                      start=(k == 0), stop=(k == KT - 1),
                                  )
                              ob = out_pool.tile([128, 256], bf16, name="ob",
                                                 tag="ob")
                              nc.scalar.activation(
                                  ob[:], wop[:],
                                  mybir.ActivationFunctionType.Identity,
                                  bias=0.0,
                                  scale=scl_sb[:, 3 + es:4 + es])
                              nc.sync.dma_start(
                                  out_c[bass.ts(es, 128),
                                        bass.ds(ci * S_CHUNK + ch2 * 256, 256)],
                                  ob[:])

                  for ci in range(N_SCHUNK):
                      sl = bass.ts(ci, S_CHUNK)
                      t_max = ci * 4 + 3 if causal else ST - 1
                      for hp in range(2):
                          pv = [pv_ps.tile([HD + 1, S_CHUNK], f32,
                                           name=f"pv{half}", tag="pv")
                                for half in range(2)]
                          for t in range(t_max + 1):
                              kslice = bass.ts(t, 128)
                              dcol = max(t * 128 - ci * S_CHUNK, 0) if causal \
                                  else 0
                              w = S_CHUNK - dcol
                              qsl = bass.ds(ci * S_CHUNK + dcol, w)
                              if use_maskt:
                                  mt = mt_pool.tile([128, S_CHUNK], f32,
                                                    name="mt", tag="mt")
                                  nc.sync.dma_start(mt[:], maskT[kslice, sl])
                              ps = qk_ps.tile([128, 2, S_CHUNK], f32,
                                              name="qk", tag="qk")
                              for half in range(2):
                                  nc.tensor.matmul(
                                      ps[:, half, dcol:],
                                      kkT[bass.ds(64 * half, 64), kslice],
                                      qT[bass.ds(64 * half, 64), hp, qsl],
                                      start=True, stop=True,
                                      tile_position=(64 * half, 0),
                                  )
                              if use_maskt:
                                  for half in range(2):
                                      nc.vector.scalar_tensor_tensor(
                                          ps[:, half, :], ps[:, half, :], 0.125,
                                          mt[:],
                                          op0=mybir.AluOpType.mult,
                                          op1=mybir.AluOpType.add)
                              ex = exp_sb.tile([128, 2, S_CHUNK], bf16,
                                               name="ex", tag="exp")
                              nc.scalar.activation(
                                  ex[:, :, dcol:], ps[:, :, dcol:],
                                  mybir.ActivationFunctionType.Exp,
                                  bias=0.0,
                                  scale=1.0 if use_maskt else 0.125)
                              if causal and t * 128 >= ci * S_CHUNK:
                                  nc.gpsimd.affine_select(
                                      ex[:, :, bass.ds(dcol, 128)],
                                      ex[:, :, bass.ds(dcol, 128)],
                                      pattern=[[0, 2], [1, 128]],
                                      compare_op=mybir.AluOpType.is_ge,
                                      fill=0.0, base=0,
                                      channel_multiplier=-1)
                              for half in range(2):
                                  nc.tensor.matmul(
                                      pv[half][:, dcol:], v_aug[:, t, :],
                                      ex[:, half, dcol:],
                                      start=(t == 0), stop=(t == t_max),
                                  )
                          for half in range(2):
                              head = hp * 2 + half
                              lrow = attn_tmp.tile([1, S_CHUNK], f32,
                                                   name="lrow", tag="lrow")
                              nc.vector.tensor_copy(lrow[:],
                                                    pv[half][HD:HD + 1, :])
                              rec = attn_tmp.tile([1, S_CHUNK], f32,
                                                  name="rec", tag="rec")
                              scr = attn_tmp.tile([1, S_CHUNK], f32,
                                                  name="scr", tag="scr")
                              nc.vector.reciprocal_approx_accurate(
                                  rec[:], lrow[:], scr[:])
                              recb = aux_ps.tile([HD, S_CHUNK], f32,
                                                 name="recb", tag="aux")
                              nc.tensor.matmul(recb[:], ones_col[:], rec[:],
                                               start=True, stop=True)
                              pvc = attn_tmp.tile([HD, S_CHUNK], f32,
                                                  name="pvc", tag="pvc")
                              nc.vector.tensor_copy(pvc[:], pv[half][0:HD, :])
                              nc.vector.tensor_mul(odT[:, head, sl], pvc[:],
                                                   recb[:])

                      # FFN + SiLU for this chunk's four heads
                      ag_in = dram.tile([ECOLS, S_CHUNK], bf16,
                                        name=f"ag_in{_rep}_{ci}",
                                        tag=f"ag_in{ci}")
                      ag_out = dram.tile([H * HD, S_CHUNK], bf16,
                                         addr_space="Shared",
                                         name=f"ag_out{_rep}_{ci}",
                                         tag=f"ag_out{_rep}_{ci}")
                      for head in range(HPC):
                          z = aux_ps.tile([HD, S_CHUNK], f32, name="z",
                                          tag="aux")
                          nc.tensor.matmul(z[:], fw_sb[:], odT[:, head, sl],
                                           start=True, stop=True)
                          at = at_pool.tile([HD, S_CHUNK], bf16, name="at",
                                            tag="at")
                          nc.scalar.activation(
                              at[:], z[:], mybir.ActivationFunctionType.Silu,
                              bias=fb_sb[:], scale=1.0)
                          nc.sync.dma_start(at_dst := ag_in[bass.ts(head, HD), :], at[:])

                      # chunked AllGather + wo for this chunk
                      nc.gpsimd.collective_compute(
                          "AllGather", mybir.AluOpType.bypass,
                          replica_groups=[list(range(N_CORES))],
                          ins=[ag_in[:].opt()], outs=[ag_out[:].opt()],
                      )
                      for _xag in range(int(_os.environ.get("XAG", "0"))):
                          xag_out = dram.tile(
                              [H * HD, S_CHUNK], bf16, addr_space="Shared",
                              name=f"xag{_rep}_{ci}_{_xag}",
                              tag=f"xag{_rep}_{ci}_{_xag}")
                          nc.gpsimd.collective_compute(
                              "AllGather", mybir.AluOpType.bypass,
                              replica_groups=[list(range(N_CORES))],
                              ins=[ag_in[:].opt()], outs=[xag_out[:].opt()],
                          )
                      pending.append((ci, ag_out))
                      if len(pending) > 1:
                          emit_wo(*pending.pop(0))
                  while pending:
                      emit_wo(*pending.pop(0))


    nc.finalize()
    return nc


def _colsteps(a, pair=False):
    """Per-column absmax/127 quantization steps; pair=True shares steps
    across adjacent column pairs (RoPE rotates those pairs together)."""
    am = np.abs(np.asarray(a, np.float32)).max(axis=0)
    if pair:
        am = np.repeat(np.maximum(am[0::2], am[1::2]), 2)
    return np.maximum(am, 1e-30) / 127.0


def _q8(a, step):
    return np.clip(np.rint(a / step), -127, 127).astype(np.int8)


def _host_prep(x, freqs_cos, freqs_sin, wq, wk, wv, wo, fw, fb):
    """Host-side layout prep: transpose/slice; wp/wo per-column int8."""
    SS = S // N_CORES
    x2 = np.asarray(x, dtype=np.float32).reshape(S, D)
    xT = x2.T.astype(BF16)                              # [D, S] bf16

    cosT = np.asarray(freqs_cos, np.float32).T          # [32, S]
    sinT = np.asarray(freqs_sin, np.float32).T
    cos64 = np.repeat(cosT, 2, axis=0)                  # [64, S]
    sin64 = np.repeat(sinT, 2, axis=0)
    sign = np.where((np.arange(HD) % 2) == 0, -1.0, 1.0).astype(np.float32)
    ss64 = sin64 * sign[:, None]
    trig_full = np.concatenate([cos64, ss64], axis=0)   # [128, S]

    fwb = np.asarray(fw, np.float32).astype(BF16)           # [d, e] natural
    fbv = np.ascontiguousarray(np.asarray(fb, np.float32).reshape(HD, 1))

    wq_f = np.asarray(wq, np.float32)
    wk_f = np.asarray(wk, np.float32)
    wv_f = np.asarray(wv, np.float32)
    wo_f = np.asarray(wo, np.float32)

    in_maps = []
    for c in range(N_CORES):
        ssl = slice(c * SS, (c + 1) * SS)
        wq_c = wq_f[:, c * ECOLS:(c + 1) * ECOLS]
        wk_c = wk_f[:, c * HD:(c + 1) * HD]
        wv_c = wv_f[:, c * HD:(c + 1) * HD]
        wpk_f = np.concatenate([wq_c, wk_c, wv_c], axis=1)
        # q and k columns rope in pairs -> pair-shared steps; v per-column
        wstep = np.concatenate([_colsteps(wpk_f[:, :320], pair=True),
                                _colsteps(wpk_f[:, 320:])])
        wpk = _q8(wpk_f, wstep[None, :])
        wo_cf = np.ascontiguousarray(wo_f[:, c * ECOLS:(c + 1) * ECOLS])
        ostep = _colsteps(wo_cf)
        wo_cc = _q8(wo_cf, ostep[None, :])
        # trig slice packed byte-wise as bf16 rows (f32 row p -> 2p, 2p+1)
        tpk = np.ascontiguousarray(trig_full[:, ssl]) \
            .view(np.uint16).reshape(2 * 128, SS).view(BF16)
        # dequant steps: [128, 5] = wp (3 groups) | wo (2 es blocks)
        sclv = np.empty((128, 5), np.float32)
        sclv[:, 0:3] = wstep.reshape(3, 128).T
        sclv[:, 3:5] = ostep.reshape(2, 128).T
        in_maps.append({
            "xs": np.concatenate(
                [np.ascontiguousarray(xT[:, ssl]), tpk], axis=0),
            "wp": np.ascontiguousarray(wpk),
            "scl": sclv,
            "fw_in": fwb, "fb_in": fbv, "wo_c": wo_cc,
        })
    return in_maps


def _build_and_prep(inputs, causal, apply_mask_t):
    """Build/cache the nc and prep per-core in_maps."""
    key = (causal, apply_mask_t)
    if key not in _nc_cache:
        _nc_cache[key] = build_nc(causal, apply_mask_t)
    in_maps = _host_prep(
        inputs["x"], inputs["freqs_cos"], inputs["freqs_sin"],
        inputs["wq"], inputs["wk"], inputs["wv"], inputs["wo"],
        inputs["fw"], inputs["fb"])
    return _nc_cache[key], in_maps


def _classify_mask(mask):
    m = np.asarray(mask, np.float32)
    if not m.any():
        return "zeros"
    tril = np.tril(np.ones((S, S), dtype=bool))
    if np.all(m[tril] == 0.0) and np.all(m[~tril] <= -1e4):
        return "causal"
    return "generic"


def _host_reference(x, freqs_cos, freqs_sin, mask, wq, wk, wv, wo, fw, fb):
    """Exact numpy fallback (pathological inputs only): matches reference()."""
    xf = np.asarray(x, np.float32).reshape(S, D)
    q = (xf @ np.asarray(wq, np.float32)).reshape(S, H, HD)
    k = (xf @ np.asarray(wk, np.float32)).reshape(S, KVH, HD)
    v = (xf @ np.asarray(wv, np.float32)).reshape(S, KVH, HD)
    cos = np.asarray(freqs_cos, np.float32)[:, None, :]
    sin = np.asarray(freqs_sin, np.float32)[:, None, :]

    def rope(t):
        tr = t.reshape(S, t.shape[1], HD // 2, 2)
        re = tr[..., 0] * cos - tr[..., 1] * sin
        im = tr[..., 0] * sin + tr[..., 1] * cos
        return np.stack([re, im], axis=-1).reshape(t.shape)

    q = rope(q)
    k = rope(k)
    k = np.repeat(k, H // KVH, axis=1)
    v = np.repeat(v, H // KVH, axis=1)
    q = q.transpose(1, 0, 2)        # (H, S, HD)
    k = k.transpose(1, 0, 2)
    v = v.transpose(1, 0, 2)
    m = np.asarray(mask, np.float32)
    out = np.empty((H, S, HD), np.float32)
    for h in range(H):
        sc = (q[h] @ k[h].T) / np.sqrt(HD).astype(np.float32) + m
        sc = sc - sc.max(axis=1, keepdims=True)
        e = np.exp(sc)
        p = e / e.sum(axis=1, keepdims=True)
        out[h] = p @ v[h]
    z = out @ np.asarray(fw, np.float32) + np.asarray(fb, np.float32)
    z = z * (1.0 / (1.0 + np.exp(-np.clip(z, -80, 80))))
    z = z.transpose(1, 0, 2).reshape(S, H * HD)
    return (z @ np.asarray(wo, np.float32)).reshape(B, S, D).astype(np.float32)


def _score_bound(x, wq, wk):
    """Rigorous upper bound on |scores|/8 via per-head row norms (RoPE is a
    per-position rotation, so it preserves these norms)."""
    xf = np.asarray(x, np.float32).reshape(S, D)
    q = xf @ np.asarray(wq, np.float32)
    k = xf @ np.asarray(wk, np.float32)
    qn = np.linalg.norm(q.reshape(S, H, HD), axis=2).max(axis=0)
    kn = np.linalg.norm(k.reshape(S, KVH, HD), axis=2).max(axis=0)
    return float((qn.reshape(KVH, H // KVH) * kn[:, None]).max() / 8.0)


def kernel(**inputs):
    x = inputs["x"]
    mask = inputs["mask"]
    kind = _classify_mask(mask)
    causal = kind == "causal"
    apply_mask_t = kind == "generic"

    # Safety: the device fast path skips softmax max-subtraction (scores are
    # tiny for this model's data distribution). Guard rigorously; fall back
    # to an exact host computation for pathological inputs.
    bound = _score_bound(x, inputs["wq"], inputs["wk"])
    mf = np.asarray(mask, np.float32)
    if apply_mask_t:
        finite_max = float(mf.max())
        row_ceiling = mf.max(axis=1)
        ok = (bound + max(finite_max, 0.0) < 80.0) and             bool((row_ceiling - bound > -80.0).all())
    else:
        ok = bound < 80.0
    if not ok:
        return _host_reference(
            x, inputs["freqs_cos"], inputs["freqs_sin"], mask,
            inputs["wq"], inputs["wk"], inputs["wv"], inputs["wo"],
            inputs["fw"], inputs["fb"])

    nc, in_maps = _build_and_prep(inputs, causal, apply_mask_t)
    if apply_mask_t:
        mT = np.ascontiguousarray(mf.T)
        for m in in_maps:
            m["maskT"] = mT

    res = run_bass_kernel_spmd(nc, in_maps, core_ids=list(range(N_CORES)))
    out = np.concatenate(
        [res.results[c]["out_c"].T.astype(np.float32)
         for c in range(N_CORES)], axis=1)
    return np.ascontiguousarray(out).reshape(B, S, D)

